# revision 29
# baseline (speedup 1.0000x reference)
"""AttnPooling kernel for 8 TRN2 NeuronCores.

Key ideas vs the naive implementation:
- Host-side token compaction: masked tokens influence nothing (their keys
  are masked in layer 1, and only the pool row survives layer 2), so only
  unmasked tokens + pool are shipped. L: 2048 -> 1536 (capacity), exact.
- 2 batch groups x 4 sequence shards (384 tokens per core).
- Layer 1: K AllGathered in two chunks, then V, overlapping Q projection,
  RoPE and score/exp work with the collectives.
- Padding/mask folded into V: pad V rows are zeroed and the softmax
  denominator comes from a keep-flag column appended to V, so exp needs no
  bias and the math matches the -inf mask exactly.
- Scores computed transposed [keys, queries] with head pairs row-packed
  into the PE array (partitions 0-63 / 64-127 run concurrently).
- Layer 2 has no big collective: pool attention is a sum over keys, so
  each core reduces over its local K2/V2 and a 4 KB AllGather + local sum
  finishes softmax. MLP2 is DFF-sharded with a second 4 KB AllGather.
"""
import contextlib

import numpy as np
import ml_dtypes

BF16 = ml_dtypes.bfloat16
B, N, D = 2, 2047, 1024
H, HD = 16, 64
NL, DFF = 2, 4096
G = 4
CAP = 1536          # padded kept length (incl pool) per batch
TLOC = CAP // G     # 384 tokens per core
NT = TLOC // 128    # 3
DT = D // 128       # 8
KT = CAP // 128     # 12
FT = DFF // 128     # 32
KSH = 4 * 128 * TLOC          # K half-shard elems (4 feature tiles)
VSH = NT * 128 * (H * 65)     # V shard elems (vaug layout: per head v|keep)
ASH = VSH + KSH               # first AG shard: V(+ones) then K half 1
SM_SH = 2048                  # small AG shard (xn_pool 1024 + xc1p 1024) bf16
AT_SH = H * 65                # attn-partial AG shard (f32)
RG = [[0, 1, 2, 3], [4, 5, 6, 7]]
EPS = 1e-5


def _bf(a):
    return np.ascontiguousarray(np.asarray(a, np.float32)).astype(BF16)


def _f32(a):
    return np.ascontiguousarray(np.asarray(a, np.float32))


def _f8(a):
    return np.ascontiguousarray(np.asarray(a, np.float32)).astype(
        ml_dtypes.float8_e4m3)


def build_program():
    import concourse.bass as bass
    import concourse.mybir as mybir
    import concourse.tile as tile

    f32 = mybir.dt.float32
    bf16 = mybir.dt.bfloat16

    nc = bass.Bass(num_devices=8)

    def din(name, shape, dt=bf16):
        return nc.declare_dram_parameter(name, shape, dt, isOutput=False)

    P = {}
    P["x_sh"] = din("x_sh", [TLOC, D], f32)
    P["ident"] = din("ident", [128, 128])
    P["cos_t"] = din("cos_t", [128, TLOC], f32)
    P["sinm_t"] = din("sinm_t", [128, TLOC], f32)
    P["keep_loc"] = din("keep_loc", [128, NT], f32)
    P["keep_lb"] = din("keep_lb", [128, NT])   # bf16 copy of keep_loc
    P["keep_f8"] = din("keep_f8", [128, NT], mybir.dt.float8e4)
    P["keep_all"] = din("keep_all", [128, KT])  # bf16, whole group
    for l in range(NL):
        for w in ("wq", "wk", "wv"):
            P[f"{w}T{l}"] = din(f"{w}T{l}", [D, D])
        P[f"bk{l}"] = din(f"bk{l}", [128, DT], f32)
        P[f"bv{l}"] = din(f"bv{l}", [1, D])
    P["bq0"] = din("bq0", [128, DT], f32)
    P["woT0"] = din("woT0", [D, D])
    P["w1T0"] = din("w1T0", [D, DFF])
    P["b1T0"] = din("b1T0", [128, FT], f32)
    P["w2T0"] = din("w2T0", [DFF, D])
    P["b20"] = din("b20", [1, D])
    P["bq1row"] = din("bq1row", [1, D], f32)
    P["wo1T"] = din("wo1T", [D, D])
    P["w1T1s"] = din("w1T1s", [D, D])
    P["b1row"] = din("b1row", [1, D], f32)
    P["w2T1s"] = din("w2T1s", [D, D])
    P["b21row"] = din("b21row", [1, D], f32)
    P["projTs"] = din("projTs", [D, 256])
    P["pbias"] = din("pbias", [1, 256], f32)
    P["out"] = nc.declare_dram_parameter("out", [1, 256], f32, isOutput=True)

    with tile.TileContext(nc) as tc:
        with contextlib.ExitStack() as es:
            _emit(nc, tc, es, P)
    _split_multiwaits(nc, mybir)
    return nc


def _split_multiwaits(nc, mybir):
    """Walrus caps sync commands on real compute ops; NoOps can hold many.
    Move multi-wait lists onto a NoOp inserted just before the instruction."""
    n = [0]

    def fresh():
        n[0] += 1
        return f"I-syncsplit-{n[0]}"

    for fn in nc.m.functions:
        for blk in fn.blocks:
            out = []
            for inst in blk.instructions:
                si = inst.sync_info
                if (si is not None and si.on_wait and len(si.on_wait) > 1
                        and type(inst).__name__ != "InstNoOp"):
                    for w in list(si.on_wait):
                        out.append(mybir.InstNoOp(
                            name=fresh(), ins=[], outs=[], engine=inst.engine,
                            sync_info=mybir.SyncInfo(on_wait=[w], on_update=[]),
                            bass_nofuse=True))
                    inst.sync_info = mybir.SyncInfo(
                        on_wait=[], on_update=list(si.on_update))
                out.append(inst)
            blk.instructions = out


def _emit(nc, tc, es, P):
    import concourse.bass as bass
    import concourse.mybir as mybir

    f32 = mybir.dt.float32
    bf16 = mybir.dt.bfloat16
    f8 = mybir.dt.float8e4
    AF = mybir.ActivationFunctionType
    OP = mybir.AluOpType
    AX = mybir.AxisListType
    ts = bass.ts
    ec = es.enter_context

    const = ec(tc.tile_pool(name="const", bufs=1))
    persist = ec(tc.tile_pool(name="persist", bufs=1))
    act = ec(tc.tile_pool(name="act", bufs=2))
    wbuf = ec(tc.tile_pool(name="wbuf", bufs=2))
    wpers = ec(tc.tile_pool(name="wpers", bufs=1))  # 8x [128,D] bf16, reused
    rope_p = ec(tc.tile_pool(name="rope", bufs=2))
    ptp = ec(tc.tile_pool(name="ptp", bufs=3))
    small = ec(tc.tile_pool(name="small", bufs=1))
    stats = ec(tc.tile_pool(name="stats", bufs=2))
    dram = ec(tc.tile_pool(name="dram", bufs=1, space="DRAM"))

    dma = nc.sync.dma_start

    def fview(dram_tile, off, p, f):
        a = dram_tile[:]
        return bass.AP(tensor=a.tensor, offset=a.offset + off,
                       ap=[[f, p], [1, f]])

    # ---------------- constants ----------------
    ident_sb = const.tile([128, 128], bf16, tag="ident", name="ident")
    dma(out=ident_sb[:], in_=P["ident"][:])
    cos_sb = const.tile([128, TLOC], f32, tag="cos", name="cos")
    dma(out=cos_sb[:], in_=P["cos_t"][:])
    sinm_sb = const.tile([128, TLOC], f32, tag="sinm", name="sinm")
    dma(out=sinm_sb[:], in_=P["sinm_t"][:])
    keep_sb = const.tile([128, NT], f32, tag="keep", name="keep")
    dma(out=keep_sb[:], in_=P["keep_loc"][:])
    eps_sb = const.tile([128, 1], f32, tag="eps", name="eps")
    nc.vector.memset(eps_sb[:], EPS)
    bq0_sb = const.tile([128, DT], f32, tag="bq0", name="bq0")
    dma(out=bq0_sb[:], in_=P["bq0"][:])
    bk_sb, bv_bc = {}, {}
    for l in range(NL):
        bk_sb[l] = const.tile([128, DT], f32, tag=f"bk{l}", name=f"bk{l}")
        dma(out=bk_sb[l][:], in_=P[f"bk{l}"][:])
        bv_bc[l] = const.tile([128, D], bf16, tag=f"bvbc{l}", name=f"bvbc{l}")
        dma(out=bv_bc[l][:], in_=P[f"bv{l}"][:].to_broadcast([128, D]))
    b1T0_sb = const.tile([128, FT], f32, tag="b1T0", name="b1T0")
    dma(out=b1T0_sb[:], in_=P["b1T0"][:])
    b2bc = const.tile([128, D], bf16, tag="b2bc", name="b2bc")
    dma(out=b2bc[:], in_=P["b20"][:].to_broadcast([128, D]))
    pbias_sb = const.tile([1, 256], f32, tag="pbias", name="pbias")
    dma(out=pbias_sb[:], in_=P["pbias"][:])
    bq1r_sb = const.tile([1, D], f32, tag="bq1r", name="bq1r")
    dma(out=bq1r_sb[:], in_=P["bq1row"][:])
    b1r_sb = const.tile([1, D], f32, tag="b1r", name="b1r")
    dma(out=b1r_sb[:], in_=P["b1row"][:])
    b21r_sb = const.tile([1, D], f32, tag="b21r", name="b21r")
    dma(out=b21r_sb[:], in_=P["b21row"][:])

    # residual stream f32, token-major
    x_res = [persist.tile([128, D], f32, tag=f"xres{t}", name=f"xres{t}")
             for t in range(NT)]
    for t in range(NT):
        dma(out=x_res[t][:], in_=P["x_sh"][ts(t, 128), :])

    # ---------------- helpers ----------------
    def ln_to_xnT(tags, psp):
        """LN of x_res (no affine; folded into weights) -> DT x [128,TLOC]
        bf16, feature-major (transposed)."""
        xnT = [persist.tile([128, TLOC], bf16, tag=tags[d], name=f"{tags[d]}_n")
               for d in range(DT)]
        for t in range(NT):
            st = stats.tile([128, 2, 6], f32, tag="bnst", name="bnst")
            nc.vector.bn_stats(out=st[:, 0, :], in_=x_res[t][:, 0:512])
            nc.vector.bn_stats(out=st[:, 1, :], in_=x_res[t][:, 512:1024])
            mv = stats.tile([128, 2], f32, tag="bnmv", name="bnmv")
            nc.vector.bn_aggr(out=mv[:], in_=st[:])
            std = stats.tile([128, 1], f32, tag="std", name="std")
            nc.scalar.activation(out=std[:], in_=mv[:, 1:2], func=AF.Sqrt,
                                 bias=eps_sb[:], scale=1.0)
            r = stats.tile([128, 1], f32, tag="rstd", name="rstd")
            nc.vector.reciprocal(out=r[:], in_=std[:])
            xsub = act.tile([128, D], f32, tag="xsub", name="xsub")
            nc.vector.tensor_scalar_sub(out=xsub[:], in0=x_res[t][:],
                                        scalar1=mv[:, 0:1])
            xn = act.tile([128, D], bf16, tag="xn", name="xn")
            nc.scalar.activation(out=xn[:], in_=xsub[:], func=AF.Identity,
                                 bias=0.0, scale=r[:])
            for d in range(DT):
                pt = psp.tile([128, 128], bf16, tag="tp", name="tp")
                nc.tensor.transpose(pt[:], xn[:, ts(d, 128)], ident_sb[:])
                nc.scalar.activation(out=xnT[d][:, ts(t, 128)], in_=pt[:],
                                     func=AF.Copy)
        return xnT

    def qk_proj(xnT, w_dram, bias_sb, tags, psp, rng=None, dest=None,
                odt=bf16):
        oT = dest if dest is not None else {}
        wa = w_dram[:]
        for e in (rng if rng is not None else range(DT)):
            if e not in oT:
                oT[e] = persist.tile([128, TLOC], odt, tag=tags[e],
                                     name=f"{tags[e]}_p")
            wc = wbuf.tile([128, DT, 128], bf16, tag="wchunk", name="wchunk")
            src = bass.AP(tensor=wa.tensor, offset=wa.offset + e * 128,
                          ap=[[D, 128], [128 * D, DT], [1, 128]])
            dma(out=wc[:], in_=src)
            pq = psp.tile([128, 512], f32, tag="pq", name="pq")
            for d in range(DT):
                nc.tensor.matmul(pq[:, 0:TLOC], wc[:, d, :], xnT[d][:],
                                 start=(d == 0), stop=(d == DT - 1))
            nc.scalar.activation(out=oT[e][:], in_=pq[:, 0:TLOC],
                                 func=AF.Identity,
                                 bias=bias_sb[:, e:e + 1], scale=1.0)
        return oT

    def v_proj(xnT, w_dram, bvbc, tags, psh):
        """-> NT tiles [128, H*65] bf16 in vaug layout (per head v(64)|keep),
        bias added, pad rows zeroed, keep column from keep_lb."""
        v = [persist.tile([128, H * 65], bf16, tag=tags[t],
                          name=f"{tags[t]}_v") for t in range(NT)]
        for hlf in range(2):
            held = [psh.tile([128, 512], f32, tag=f"vh{i}", name=f"vh{i}")
                    for i in range(NT)]
            for d in range(DT):
                wvc = wbuf.tile([128, 512], bf16, tag="wvchunk", name="wvchunk")
                dma(out=wvc[:], in_=w_dram[ts(d, 128), ts(hlf, 512)])
                for t in range(NT):
                    nc.tensor.matmul(held[t][:], xnT[d][:, ts(t, 128)], wvc[:],
                                     start=(d == 0), stop=(d == DT - 1),
                                     skip_group_check=True)
            for t in range(NT):
                a = v[t][:]
                vout = bass.AP(tensor=a.tensor,
                               offset=a.offset + hlf * 8 * 65,
                               ap=[a.ap[0], [65, 8], [1, 64]])
                nc.vector.tensor_tensor(out=vout, in0=held[t][:],
                                        in1=bvbc[:, ts(hlf, 512)], op=OP.add)
                nc.vector.tensor_scalar_mul(out=vout, in0=vout,
                                            scalar1=keep_sb[:, t:t + 1])
        for t in range(NT):
            a = v[t][:]
            ones_dst = bass.AP(tensor=a.tensor, offset=a.offset + 64,
                               ap=[a.ap[0], [65, H]])
            kb = P["keep_lb"][:, t:t + 1]
            dma(out=ones_dst, in_=bass.AP(tensor=kb.tensor, offset=kb.offset,
                                          ap=[kb.ap[0], [0, H]]))
        return v

    def rope(q):
        shuf = rope_p.tile([128, TLOC], bf16, tag="shuf", name="shuf")
        for blk in range(2):
            b0 = 64 * blk
            nc.vector.tensor_copy(out=shuf[b0:b0 + 32, :],
                                  in_=q[b0 + 32:b0 + 64, :])
            nc.vector.tensor_copy(out=shuf[b0 + 32:b0 + 64, :],
                                  in_=q[b0:b0 + 32, :])
        qc = rope_p.tile([128, TLOC], bf16, tag="qcos", name="qcos")
        nc.vector.tensor_tensor(out=qc[:], in0=q[:], in1=cos_sb[:], op=OP.mult)
        qs = rope_p.tile([128, TLOC], bf16, tag="qsin", name="qsin")
        nc.vector.tensor_tensor(out=qs[:], in0=shuf[:], in1=sinm_sb[:],
                                op=OP.mult)
        nc.vector.tensor_tensor(out=q[:], in0=qc[:], in1=qs[:], op=OP.add)

    # ================= LAYER 1 =================
    xnT_tags = [f"xnT{d}" for d in range(DT)]
    qT_tags = [f"qT{d}" for d in range(DT)]
    kT_tags = [f"kT{d}" for d in range(DT)]
    vg_tags = [f"vg{t}" for t in range(NT)]

    with tc.tile_pool(name="ps_a", bufs=2, space="PSUM") as ps_a, \
         tc.tile_pool(name="ps_b", bufs=1, space="PSUM") as ps_b:
        xnT = ln_to_xnT(xnT_tags, ps_a)

        # V (vaug layout) + K half 1 in one AG, K half 2 in a second.
        v_sb = v_proj(xnT, P["wvT0"], bv_bc[0], vg_tags, ps_b)
        ag_a_in = dram.tile([ASH], bf16, tag="agai1", name="agai1")
        ag_a_out = dram.tile([G * ASH], bf16, tag="agao1", name="agao1")
        for t in range(NT):
            dma(out=fview(ag_a_in, t * 128 * H * 65, 128, H * 65),
                in_=v_sb[t][:])
        kT = {}
        qk_proj(xnT, P["wkT0"], bk_sb[0], kT_tags, ps_a, range(4), dest=kT)
        for e in range(4):
            rope(kT[e])
            dma(out=fview(ag_a_in, VSH + e * 128 * TLOC, 128, TLOC),
                in_=kT[e][:])
        nc.gpsimd.collective_compute(
            "AllGather", OP.bypass, replica_groups=RG,
            ins=[ag_a_in[:]], outs=[ag_a_out[:]])

        ag_b_in = dram.tile([KSH], bf16, tag="agbi", name="agbi")
        ag_b_out = dram.tile([G * KSH], bf16, tag="agbo", name="agbo")
        qk_proj(xnT, P["wkT0"], bk_sb[0], kT_tags, ps_a, range(4, 8), dest=kT)
        for e in range(4, 8):
            rope(kT[e])
            dma(out=fview(ag_b_in, (e - 4) * 128 * TLOC, 128, TLOC),
                in_=kT[e][:])
        nc.gpsimd.collective_compute(
            "AllGather", OP.bypass, replica_groups=RG,
            ins=[ag_b_in[:]], outs=[ag_b_out[:]])

        qT = qk_proj(xnT, P["wqT0"], bq0_sb, qT_tags, ps_a)
        for e in range(DT):
            rope(qT[e])

        # khd: [128, CAP] bf16 per feature tile; one 3D DMA each
        khd = []
        for dt_ in range(DT):
            t_ = persist.tile([128, CAP], bf16, tag=f"khd{dt_}",
                              name=f"khd{dt_}")
            khd.append(t_)
            if dt_ < 4:
                ao = ag_a_out[:]
                src = bass.AP(
                    tensor=ao.tensor,
                    offset=ao.offset + VSH + dt_ * 128 * TLOC,
                    ap=[[TLOC, 128], [ASH, G], [1, TLOC]])
            else:
                ao = ag_b_out[:]
                src = bass.AP(
                    tensor=ao.tensor,
                    offset=ao.offset + (dt_ - 4) * 128 * TLOC,
                    ap=[[TLOC, 128], [KSH, G], [1, TLOC]])
            dst = bass.AP(tensor=t_[:].tensor, offset=t_[:].offset,
                          ap=[t_[:].ap[0], [TLOC, G], [1, TLOC]])
            dma(out=dst, in_=src)

        # vaug[k]: contiguous copies from the gathered vaug-layout V
        vaug = []
        for k in range(KT):
            va = persist.tile([128, H * 65], bf16, tag=f"vg_a{k}",
                              name=f"vg_a{k}")
            vaug.append(va)
            r, lt = k // NT, k % NT
            dma(out=va[:],
                in_=fview(ag_a_out, r * ASH + lt * 128 * H * 65,
                          128, H * 65))

    # attention: scores transposed [keys, queries], head pairs row-packed
    oT = [persist.tile([128, TLOC], bf16, tag=xnT_tags[d], name=f"oT{d}")
          for d in range(DT)]
    # dens live 4-per-tile at partition starts {0,32,64,96} (engine ops
    # require 32-aligned start partitions)
    den_sb = [small.tile([128, TLOC], f32, tag=f"den{i}", name=f"den{i}")
              for i in range(4)]
    for i in range(4):
        nc.vector.memset(den_sb[i][:], 1.0)
    with tc.tile_pool(name="ps_sc", bufs=2, space="PSUM") as ps_sc, \
         tc.tile_pool(name="ps_pav", bufs=2, space="PSUM") as ps_pav:
        for dt_ in range(DT):
            pav = [ps_pav.tile([65, 512], f32, tag=f"pav{hh}",
                               name=f"pav{hh}_{dt_}") for hh in range(2)]
            pts = {}

            def _av(k):
                for hh in range(2):
                    h = 2 * dt_ + hh
                    nc.tensor.matmul(pav[hh][0:65, 0:TLOC],
                                     vaug[k][:, 65 * h:65 * h + 65],
                                     pts[k][:, hh, :],
                                     start=(k == 0), stop=(k == KT - 1),
                                     skip_group_check=True)

            for k in range(KT):
                sc = ps_sc.tile([128, 2, 512], f32, tag="sc", name="sc")
                for hh in range(2):
                    off = 64 * hh
                    nc.tensor.matmul(sc[:, hh, 0:TLOC],
                                     khd[dt_][off:off + 64, ts(k, 128)],
                                     qT[dt_][off:off + 64, :],
                                     start=True, stop=True,
                                     skip_group_check=True)
                pt = ptp.tile([128, 2, TLOC], bf16, tag="pt", name="pt")
                nc.scalar.activation(out=pt[:], in_=sc[:, :, 0:TLOC],
                                     func=AF.Exp, bias=0.0, scale=0.125)
                pts[k] = pt
                # pipeline: AV trails scores by one k so the in-order PE
                # queue never waits on the exp of the current tile
                if k >= 1:
                    _av(k - 1)
            _av(KT - 1)
            for hh in range(2):
                h = 2 * dt_ + hh
                off = 64 * hh
                nc.vector.tensor_copy(out=oT[dt_][off:off + 64, :],
                                      in_=pav[hh][0:64, 0:TLOC])
                dp = 32 * (h % 4)
                nc.vector.tensor_copy(
                    out=den_sb[h // 4][dp:dp + 1, :],
                    in_=pav[hh][64:65, 0:TLOC])

    invd_d = dram.tile([H * TLOC], f32, tag="invd_d", name="invd_d")
    for i in range(4):
        inv4 = small.tile([128, TLOC], f32, tag="inv4", name=f"inv4_{i}")
        nc.vector.reciprocal(out=inv4[:], in_=den_sb[i][:])
        a = inv4[:]
        src = bass.AP(tensor=a.tensor, offset=a.offset,
                      ap=[[a.ap[0][0] * 32, 4], [1, TLOC]])
        dma(out=fview(invd_d, 4 * i * TLOC, 4, TLOC), in_=src)
    for dt_ in range(DT):
        bc = act.tile([128, TLOC], f32, tag="invdbc", name="invdbc")
        ia = invd_d[:]
        src = bass.AP(tensor=ia.tensor, offset=ia.offset + 2 * dt_ * TLOC,
                      ap=[[TLOC, 2], [0, 64], [1, TLOC]])
        dma(out=bc[:], in_=src)
        nc.vector.tensor_tensor(out=oT[dt_][:], in0=oT[dt_][:], in1=bc[:],
                                op=OP.mult)

    # O-projection + residual
    w8 = [wpers.tile([128, D], bf16, tag=f"w8_{d}", name=f"wo8_{d}")
          for d in range(DT)]
    for d in range(DT):
        dma(out=w8[d][:], in_=P["woT0"][ts(d, 128), :])
    with tc.tile_pool(name="ps_c", bufs=2, space="PSUM") as ps_c:
        for t in range(NT):
            for hlf in range(2):
                po = ps_c.tile([128, 512], f32, tag="po", name="po")
                for d in range(DT):
                    nc.tensor.matmul(po[:], oT[d][:, ts(t, 128)],
                                     w8[d][:, ts(hlf, 512)],
                                     start=(d == 0), stop=(d == DT - 1))
                nc.vector.tensor_tensor(out=x_res[t][:, ts(hlf, 512)],
                                        in0=x_res[t][:, ts(hlf, 512)],
                                        in1=po[:], op=OP.add)

    # tail-weight prefetch into now-dead SBUF: wq1 -> w8 (during MLP1),
    # w1T1s -> khd slots, w2T1s -> vg_a slots, proj -> its own tiles
    for d in range(DT):
        dma(out=w8[d][:], in_=P["wqT1"][ts(d, 128), :])
    w1s_sb = [persist.tile([128, 512], bf16, tag=f"khd{d}", name=f"w1s{d}")
              for d in range(DT)]
    w1s_sb += [persist.tile([128, 512], bf16, tag=f"vg_a{k}",
                            name=f"w1sb{k}") for k in range(3, 11)]
    for d in range(DT):
        dma(out=w1s_sb[d][:], in_=P["w1T1s"][ts(d, 128), 0:512])
        dma(out=w1s_sb[8 + d][:], in_=P["w1T1s"][ts(d, 128), 512:1024])

    proj_sb = [wbuf.tile([128, 256], bf16, tag=f"projc{d}", name=f"projc{d}")
               for d in range(DT)]
    for d in range(DT):
        dma(out=proj_sb[d][:], in_=P["projTs"][ts(d, 128), :])

    with tc.tile_pool(name="ps_d", bufs=2, space="PSUM") as ps_d:
        xn2T = ln_to_xnT(qT_tags, ps_d)  # reuse qT slots (dead)
    for t in range(NT):  # pre-add the MLP output bias while PE runs MLP
        nc.vector.tensor_tensor(out=x_res[t][:], in0=x_res[t][:],
                                in1=b2bc[:], op=OP.add)

    with tc.tile_pool(name="ps_mlp", bufs=2, space="PSUM") as ps_mlp, \
         tc.tile_pool(name="ps_hld", bufs=1, space="PSUM") as ps_hld:
        held = [ps_hld.tile([128, 2, 512], f32, tag=f"mh{t}", name=f"mh{t}")
                for t in range(NT)]
        hTs, w2cs = {}, {}

        def _mlp2nd(f):
            for t in range(NT):
                for hlf in range(2):
                    nc.tensor.matmul(held[t][:, hlf, :], hTs[f][:, ts(t, 128)],
                                     w2cs[f][:, ts(hlf, 512)],
                                     start=(f == 0), stop=(f == FT - 1),
                                     skip_group_check=True)

        for f in range(FT):
            wc = wbuf.tile([128, DT, 128], bf16, tag="w1chunk", name="w1chunk")
            wa = P["w1T0"][:]
            src = bass.AP(tensor=wa.tensor, offset=wa.offset + f * 128,
                          ap=[[DFF, 128], [128 * DFF, DT], [1, 128]])
            dma(out=wc[:], in_=src)
            ph = ps_mlp.tile([128, 512], f32, tag="ph", name="ph")
            for d in range(DT):
                nc.tensor.matmul(ph[:, 0:TLOC], wc[:, d, :], xn2T[d][:],
                                 start=(d == 0), stop=(d == DT - 1))
            hT = act.tile([128, TLOC], bf16, tag="hT", name="hT")
            nc.scalar.activation(out=hT[:], in_=ph[:, 0:TLOC], func=AF.Gelu,
                                 bias=b1T0_sb[:, f:f + 1], scale=1.0)
            hTs[f] = hT
            w2c = wbuf.tile([128, D], bf16, tag="w2chunk", name="w2chunk")
            dma(out=w2c[:], in_=P["w2T0"][ts(f, 128), :])
            w2cs[f] = w2c
            if f >= 1:
                _mlp2nd(f - 1)
        _mlp2nd(FT - 1)
        for t in range(NT):
            for hlf in range(2):
                nc.vector.tensor_tensor(out=x_res[t][:, ts(hlf, 512)],
                                        in0=x_res[t][:, ts(hlf, 512)],
                                        in1=held[t][:, hlf, :], op=OP.add)

    # ================= LAYER 2 =================
    with tc.tile_pool(name="ps_e", bufs=2, space="PSUM") as ps_e:
        xn3T = ln_to_xnT(xnT_tags, ps_e)

        # small AG: pool xn (feature-major col 0) + pool residual row, bf16
        xc1p_bf = small.tile([1, D], bf16, tag="rb_a", name="xc1pbf")
        nc.scalar.activation(out=xc1p_bf[:], in_=x_res[0][0:1, :], func=AF.Copy)
        ag_s_in = dram.tile([SM_SH], bf16, tag="agsi", name="agsi")
        ag_s_out = dram.tile([G * SM_SH], bf16, tag="agso", name="agso")
        for d in range(DT):
            dma(out=fview(ag_s_in, d * 128, 128, 1), in_=xn3T[d][:, 0:1])
        dma(out=fview(ag_s_in, D, 1, D), in_=xc1p_bf[:])
        nc.gpsimd.collective_compute(
            "AllGather", OP.bypass, replica_groups=RG,
            ins=[ag_s_in[:]], outs=[ag_s_out[:]])

        kT2 = qk_proj(xn3T, P["wkT1"], bk_sb[1], kT_tags, ps_e)
        for e in range(DT):
            rope(kT2[e])

    with tc.tile_pool(name="ps_f", bufs=1, space="PSUM") as ps_f:
        v2 = v_proj(xn3T, P["wvT1"], bv_bc[1], vg_tags, ps_f)

    vaug2 = v2  # v_proj already emits the vaug layout with keep column

    with tc.tile_pool(name="ps_g", bufs=1, space="PSUM") as ps:
        # read back pool xn + residual from shard 0 of small AG
        xnp = small.tile([128, DT], bf16, tag="xnp", name="xnp")
        ao = ag_s_out[:]
        dma(out=xnp[:], in_=bass.AP(tensor=ao.tensor, offset=ao.offset,
                                    ap=[[1, 128], [128, DT]]))
        xc1p = small.tile([1, D], f32, tag="rf_a", name="xc1p")
        xc1p_b2 = small.tile([1, D], bf16, tag="rb_b", name="xc1pb2")
        dma(out=xc1p_b2[:], in_=bass.AP(tensor=ao.tensor,
                                        offset=ao.offset + D,
                                        ap=[[1, 1], [1, D]]))
        nc.vector.tensor_copy(out=xc1p[:], in_=xc1p_b2[:])

        # q2 row = xnp.T @ Wq1T + bias (w8 holds prefetched Wq1)
        rps = ps.tile([1, D], f32, tag="rps", name="q2ps")
        for hlf in range(2):
            for d in range(DT):
                nc.tensor.matmul(rps[:, ts(hlf, 512)], xnp[:, d:d + 1],
                                 w8[d][:, ts(hlf, 512)],
                                 start=(d == 0), stop=(d == DT - 1),
                                 skip_group_check=True)
        for d in range(DT):  # wo1 -> w8; overlaps partial attention
            dma(out=w8[d][:], in_=P["wo1T"][ts(d, 128), :])
        q2row = small.tile([1, D], bf16, tag="rb_c", name="q2row")
        q2f = small.tile([1, D], f32, tag="rf_b", name="q2f")
        nc.vector.tensor_tensor(out=q2f[:], in0=rps[:], in1=bq1r_sb[:],
                                op=OP.add)
        nc.vector.tensor_copy(out=q2row[:], in_=q2f[:])
        q2_d = dram.tile([D], bf16, tag="q2d", name="q2d")
        dma(out=fview(q2_d, 0, 1, D), in_=q2row[:])
        q2T = small.tile([128, DT], bf16, tag="q2T", name="q2T")
        qd = q2_d[:]
        dma(out=q2T[:], in_=bass.AP(tensor=qd.tensor, offset=qd.offset,
                                    ap=[[1, 128], [128, DT]]))

        # partial pool attention over local keys
        p2ps = ps.tile([128, 2 * NT * DT], f32, tag="p2ps", name="p2ps")
        for dt_ in range(DT):
            for k in range(NT):
                for hh in range(2):
                    off = 64 * hh
                    c = dt_ * 2 * NT + hh * NT + k
                    nc.tensor.matmul(p2ps[:, c:c + 1],
                                     kT2[dt_][off:off + 64, ts(k, 128)],
                                     q2T[off:off + 64, dt_:dt_ + 1],
                                     start=True, stop=True,
                                     skip_group_check=True)
        p2sb = small.tile([128, 2 * NT * DT], bf16, tag="p2sb", name="p2sb")
        nc.scalar.activation(out=p2sb[:], in_=p2ps[:], func=AF.Exp,
                             bias=0.0, scale=0.125)
        # h outer / k inner: accumulation groups must be sequential within a
        # PSUM bank (start=True clears the whole bank's has_written bits)
        o2ps = ps.tile([65, 16], f32, tag="o2ps", name="o2ps")
        for h in range(H):
            for k in range(NT):
                c = (h // 2) * 2 * NT + (h % 2) * NT + k
                nc.tensor.matmul(o2ps[:, h:h + 1],
                                 vaug2[k][:, 65 * h:65 * h + 65],
                                 p2sb[:, c:c + 1],
                                 start=(k == 0), stop=(k == NT - 1),
                                 skip_group_check=True)
        # partials -> AG -> sum
        part_sb = small.tile([65, 16], f32, tag="part", name="part")
        nc.vector.tensor_copy(out=part_sb[:], in_=o2ps[:])
        ag_a_in = dram.tile([AT_SH], f32, tag="agai", name="agai")
        ag_a_out = dram.tile([G * AT_SH], f32, tag="agao", name="agao")
        dma(out=fview(ag_a_in, 0, 65, 16), in_=part_sb[:])
        nc.gpsimd.collective_compute(
            "AllGather", OP.bypass, replica_groups=RG,
            ins=[ag_a_in[:]], outs=[ag_a_out[:]])
        sums = small.tile([65, G, 16], f32, tag="sums", name="sums")
        for r in range(G):
            dma(out=sums[:, r, :], in_=fview(ag_a_out, r * AT_SH, 65, 16))
        tot = small.tile([65, 16], f32, tag="tot", name="tot")
        nc.vector.tensor_tensor(out=sums[:, 0, :], in0=sums[:, 0, :],
                                in1=sums[:, 1, :], op=OP.add)
        nc.vector.tensor_tensor(out=sums[:, 2, :], in0=sums[:, 2, :],
                                in1=sums[:, 3, :], op=OP.add)
        nc.vector.tensor_tensor(out=tot[:], in0=sums[:, 0, :],
                                in1=sums[:, 2, :], op=OP.add)
        # o2 feature-major [128, DT] f32 via 2 DMAs; divide -> bf16
        o2f = small.tile([128, DT], f32, tag="o2f", name="o2f")
        ta = tot[:]
        for a in range(2):
            dma(out=o2f[64 * a:64 * a + 64, :],
                in_=bass.AP(tensor=ta.tensor, offset=ta.offset + a,
                            ap=[[ta.ap[0][0], 64], [2, DT]]))
        den2 = small.tile([1, H], f32, tag="den2", name="den2")
        nc.vector.tensor_copy(out=den2[:], in_=tot[64:65, :])
        invd2 = small.tile([1, H], f32, tag="invd2", name="invd2")
        nc.vector.reciprocal(out=invd2[:], in_=den2[:])
        den_d = dram.tile([H], f32, tag="den_d", name="den_d")
        dma(out=fview(den_d, 0, 1, H), in_=invd2[:])
        o2bc = small.tile([128, DT], f32, tag="o2bc", name="o2bc")
        dd = den_d[:]
        for a in range(2):
            dma(out=o2bc[64 * a:64 * a + 64, :],
                in_=bass.AP(tensor=dd.tensor, offset=dd.offset + a,
                            ap=[[0, 64], [2, DT]]))
        o2sb = small.tile([128, DT], bf16, tag="o2sb", name="o2sb")
        nc.vector.tensor_tensor(out=o2sb[:], in0=o2f[:], in1=o2bc[:],
                                op=OP.mult)

        # x2 row = xc1p + o2 @ Wo2 (w8 holds prefetched Wo1)
        rps2 = ps.tile([1, D], f32, tag="rps", name="x2ps")
        for hlf in range(2):
            for d in range(DT):
                nc.tensor.matmul(rps2[:, ts(hlf, 512)], o2sb[:, d:d + 1],
                                 w8[d][:, ts(hlf, 512)],
                                 start=(d == 0), stop=(d == DT - 1),
                                 skip_group_check=True)
        for d in range(DT):  # w2T1s -> w8; overlaps rowLN/gelu below
            dma(out=w8[d][:], in_=P["w2T1s"][ts(d, 128), :])
        x2row = small.tile([1, D], f32, tag="rf_c", name="x2row")
        nc.vector.tensor_tensor(out=x2row[:], in0=xc1p[:], in1=rps2[:],
                                op=OP.add)

        def row_ln(xrow, out_tag, nm):
            """LN of a [1, D] f32 row -> [1, D] bf16 (no affine)."""
            s1 = small.tile([1, 1], f32, tag="lns1", name=f"{nm}s1")
            nc.vector.reduce_sum(out=s1[:], in_=xrow[:], axis=AX.X)
            mean = small.tile([1, 1], f32, tag="lnmean", name=f"{nm}mean")
            nc.vector.tensor_scalar_mul(out=mean[:], in0=s1[:],
                                        scalar1=1.0 / D)
            xc = small.tile([1, D], f32, tag="rf_d", name=f"{nm}xc")
            nc.vector.tensor_scalar_sub(out=xc[:], in0=xrow[:],
                                        scalar1=mean[:])
            sq = small.tile([1, D], f32, tag="rf_e", name=f"{nm}sq")
            nc.vector.tensor_tensor(out=sq[:], in0=xc[:], in1=xc[:],
                                    op=OP.mult)
            s2 = small.tile([1, 1], f32, tag="lns2", name=f"{nm}s2")
            nc.vector.reduce_sum(out=s2[:], in_=sq[:], axis=AX.X)
            std = small.tile([1, 1], f32, tag="lnstd", name=f"{nm}sd")
            nc.scalar.activation(out=std[:], in_=s2[:], func=AF.Sqrt,
                                 bias=eps_sb[0:1, :], scale=1.0 / D)
            rr = small.tile([1, 1], f32, tag="lnrr", name=f"{nm}rr")
            nc.vector.reciprocal(out=rr[:], in_=std[:])
            xo = small.tile([1, D], bf16, tag=out_tag, name=f"{nm}o")
            nc.scalar.activation(out=xo[:], in_=xc[:], func=AF.Identity,
                                 bias=0.0, scale=rr[:])
            return xo

        def row_to_fmaj(row_bf, tag, nm):
            """[1, D] bf16 row -> [128, DT] bf16 feature-major via DRAM."""
            rd = dram.tile([D], bf16, tag=f"{tag}_d", name=f"{nm}_d")
            dma(out=fview(rd, 0, 1, D), in_=row_bf[:])
            fm = small.tile([128, DT], bf16, tag=tag, name=nm)
            a = rd[:]
            dma(out=fm[:], in_=bass.AP(tensor=a.tensor, offset=a.offset,
                                       ap=[[1, 128], [128, DT]]))
            return fm

        xn2f = row_ln(x2row, "rb_d", "ln2f")
        xn2fm = row_to_fmaj(xn2f, "fm_a", "xn2fm")

        # sharded MLP2 (this core's 1024 DFF rows), weights prefetched
        hps = ps.tile([1, D], f32, tag="rps", name="hps")
        for hlf in range(2):
            for d in range(DT):
                nc.tensor.matmul(hps[:, ts(hlf, 512)], xn2fm[:, d:d + 1],
                                 w1s_sb[8 * hlf + d][:],
                                 start=(d == 0), stop=(d == DT - 1),
                                 skip_group_check=True)
        hrow_f = small.tile([1, D], f32, tag="rf_b", name="hrowf")
        nc.vector.tensor_tensor(out=hrow_f[:], in0=hps[:], in1=b1r_sb[:],
                                op=OP.add)
        hrow = small.tile([1, D], bf16, tag="rb_c", name="hrow")
        nc.scalar.activation(out=hrow[:], in_=hrow_f[:], func=AF.Gelu,
                             bias=0.0, scale=1.0)
        hfm = row_to_fmaj(hrow, "fm_b", "hfm")
        yps = ps.tile([1, D], f32, tag="rps", name="yps")
        for hlf in range(2):
            for d in range(DT):
                nc.tensor.matmul(yps[:, ts(hlf, 512)], hfm[:, d:d + 1],
                                 w8[d][:, ts(hlf, 512)],
                                 start=(d == 0), stop=(d == DT - 1),
                                 skip_group_check=True)
        y2row = small.tile([1, D], f32, tag="rf_b", name="y2row")
        nc.vector.tensor_copy(out=y2row[:], in_=yps[:])
        ag_m_in = dram.tile([D], f32, tag="agmi", name="agmi")
        ag_m_out = dram.tile([G * D], f32, tag="agmo", name="agmo")
        dma(out=fview(ag_m_in, 0, 1, D), in_=y2row[:])
        nc.gpsimd.collective_compute(
            "AllGather", OP.bypass, replica_groups=RG,
            ins=[ag_m_in[:]], outs=[ag_m_out[:]])
        yacc = small.tile([1, D], f32, tag="rf_d", name="yacc")
        dma(out=yacc[:], in_=fview(ag_m_out, 0, 1, D))
        for r in range(1, G):
            ypart = small.tile([1, D], f32, tag="rf_e", name=f"ypart{r}")
            dma(out=ypart[:], in_=fview(ag_m_out, r * D, 1, D))
            nc.vector.tensor_tensor(out=yacc[:], in0=yacc[:], in1=ypart[:],
                                    op=OP.add)
        x3row = small.tile([1, D], f32, tag="rf_a", name="x3row")
        nc.vector.tensor_tensor(out=x3row[:], in0=x2row[:], in1=yacc[:],
                                op=OP.add)
        nc.vector.tensor_tensor(out=x3row[:], in0=x3row[:], in1=b21r_sb[:],
                                op=OP.add)

        xn3 = row_ln(x3row, "rb_d", "ln3")
        xn3fm = row_to_fmaj(xn3, "fm_a", "xn3fm")
        pps = ps.tile([1, 256], f32, tag="pps", name="pps")
        for d in range(DT):
            nc.tensor.matmul(pps[:], xn3fm[:, d:d + 1], proj_sb[d][:],
                             start=(d == 0), stop=(d == DT - 1),
                             skip_group_check=True)
        outsb = small.tile([1, 256], f32, tag="rf_e", name="outsb")
        nc.vector.tensor_tensor(out=outsb[:], in0=pps[:], in1=pbias_sb[:],
                                op=OP.add)
        dma(out=P["out"][:], in_=outsb[:])


def _host_prep(inputs):
    x = _f32(inputs["x"])
    mask = np.asarray(inputs["attention_mask"])
    pool = _f32(inputs["pool_token"]).reshape(1, D)

    Wq, Wk, Wv, Wo = (_f32(inputs[k]) for k in ("Wq", "Wk", "Wv", "Wo"))
    g1, b1l = _f32(inputs["ln1_g"]), _f32(inputs["ln1_b"])
    g2, b2l = _f32(inputs["ln2_g"]), _f32(inputs["ln2_b"])
    W1, b1 = _f32(inputs["W1"]), _f32(inputs["b1"])
    W2, b2 = _f32(inputs["W2"]), _f32(inputs["b2"])
    outg, outb = _f32(inputs["out_g"]), _f32(inputs["out_b"])
    pW, pb = _f32(inputs["proj_W"]), _f32(inputs["proj_b"])

    com = {"ident": _bf(np.eye(128))}
    for l in range(NL):
        com[f"wqT{l}"] = _bf((Wq[l] * g1[l][None, :]).T)
        com[f"wkT{l}"] = _bf((Wk[l] * g1[l][None, :]).T)
        com[f"wvT{l}"] = _bf((Wv[l] * g1[l][None, :]).T)
        com[f"bk{l}"] = _f32((b1l[l] @ Wk[l].T).reshape(DT, 128).T)
        com[f"bv{l}"] = _bf((b1l[l] @ Wv[l].T).reshape(1, D))
    com["bq0"] = _f32((b1l[0] @ Wq[0].T).reshape(DT, 128).T)
    com["woT0"] = _bf(Wo[0].T)
    com["w1T0"] = _bf((W1[0] * g2[0][None, :]).T)
    com["b1T0"] = _f32((b1[0] + b2l[0] @ W1[0].T).reshape(FT, 128).T)
    com["w2T0"] = _bf(W2[0].T)
    com["b20"] = _bf(b2[0].reshape(1, D))
    com["bq1row"] = _f32((b1l[1] @ Wq[1].T).reshape(1, D))
    com["wo1T"] = _bf(Wo[1].T)
    com["b21row"] = _f32(b2[1].reshape(1, D))
    proj_eff = pW * outg[None, :]
    pbias_full = outb @ pW.T + pb
    b1_full_l2 = b1[1] + b2l[1] @ W1[1].T
    w1eff_l2 = W1[1] * g2[1][None, :]

    inv = 10000.0 ** (-np.arange(0, HD, 2, dtype=np.float64) / HD)
    posg = np.arange(N + 1, dtype=np.float64)
    ang = posg[None, :] * inv[:, None]
    cosl, sinl = np.cos(ang), np.sin(ang)
    cosl[:, 0], sinl[:, 0] = 1.0, 0.0
    cos64 = np.concatenate([cosl, cosl], 0)
    sinm64 = np.concatenate([-sinl, sinl], 0)
    cos128 = _f32(np.concatenate([cos64, cos64], 0))      # [128, N+1]
    sinm128 = _f32(np.concatenate([sinm64, sinm64], 0))

    in_maps = []
    for core in range(8):
        g, j = core // G, core % G
        kept_pos = np.concatenate(
            [[0], 1 + np.nonzero(mask[g] != 0)[0]]).astype(np.int64)
        nk = len(kept_pos)
        assert nk <= CAP, f"kept {nk} exceeds capacity {CAP}"
        pos_pad = np.zeros(CAP, np.int64)
        pos_pad[:nk] = kept_pos
        keep = np.zeros(CAP, np.float32)
        keep[:nk] = 1.0
        xcg = np.concatenate([pool, x[g]], axis=0)        # [N+1, D]
        xc_kept = np.zeros((CAP, D), np.float32)
        xc_kept[:nk] = xcg[kept_pos]

        sl = slice(j * TLOC, (j + 1) * TLOC)
        d = dict(com)
        d["x_sh"] = _f32(xc_kept[sl])
        d["cos_t"] = _f32(cos128[:, pos_pad[sl]])
        d["sinm_t"] = _f32(sinm128[:, pos_pad[sl]])
        d["keep_loc"] = _f32(keep[sl].reshape(NT, 128).T)
        d["keep_lb"] = _bf(keep[sl].reshape(NT, 128).T)
        d["keep_f8"] = keep[sl].reshape(NT, 128).T.astype(
            ml_dtypes.float8_e4m3)
        d["keep_all"] = _bf(keep.reshape(KT, 128).T)
        dffsl = slice(j * 1024, (j + 1) * 1024)
        d["w1T1s"] = _bf(w1eff_l2[dffsl, :].T)
        d["b1row"] = _f32(b1_full_l2[dffsl].reshape(1, D))
        d["w2T1s"] = _bf(W2[1][:, dffsl].T)
        osl = slice(j * 256, (j + 1) * 256)
        d["projTs"] = _bf(proj_eff[osl, :].T)
        d["pbias"] = _f32(pbias_full[osl].reshape(1, 256))
        in_maps.append(d)
    return in_maps


_PROGRAM = None
LAST = None  # last BassKernelResults (for test.py profiling)


def kernel(**inputs):
    global _PROGRAM, LAST
    from concourse.bass_utils import run_bass_kernel_spmd
    in_maps = _host_prep(inputs)
    if _PROGRAM is None:
        _PROGRAM = build_program()
    LAST = run_bass_kernel_spmd(_PROGRAM, in_maps, list(range(8)))
    res = LAST.results
    out = np.zeros((B, D), np.float32)
    for core in range(8):
        g, j = core // G, core % G
        out[g, j * 256:(j + 1) * 256] = np.asarray(
            res[core]["out"], np.float32).reshape(256)
    return out


# revision 30
# speedup vs baseline: 1.0370x; 1.0370x over previous
"""AttnPooling kernel for 8 TRN2 NeuronCores.

Key ideas vs the naive implementation:
- Host-side token compaction: masked tokens influence nothing (their keys
  are masked in layer 1, and only the pool row survives layer 2), so only
  unmasked tokens + pool are shipped. L: 2048 -> 1536 (capacity), exact.
- 2 batch groups x 4 sequence shards (384 tokens per core).
- Layer 1: K AllGathered in two chunks, then V, overlapping Q projection,
  RoPE and score/exp work with the collectives.
- Padding/mask folded into V: pad V rows are zeroed and the softmax
  denominator comes from a keep-flag column appended to V, so exp needs no
  bias and the math matches the -inf mask exactly.
- Scores computed transposed [keys, queries] with head pairs row-packed
  into the PE array (partitions 0-63 / 64-127 run concurrently).
- Layer 2 has no big collective: pool attention is a sum over keys, so
  each core reduces over its local K2/V2 and a 4 KB AllGather + local sum
  finishes softmax. MLP2 is DFF-sharded with a second 4 KB AllGather.
"""
import contextlib

import numpy as np
import ml_dtypes

BF16 = ml_dtypes.bfloat16
B, N, D = 2, 2047, 1024
H, HD = 16, 64
NL, DFF = 2, 4096
G = 4
CAP = 1536          # padded kept length (incl pool) per batch
TLOC = CAP // G     # 384 tokens per core
NT = TLOC // 128    # 3
DT = D // 128       # 8
KT = CAP // 128     # 12
FT = DFF // 128     # 32
KSH = 4 * 128 * TLOC          # K half-shard elems (4 feature tiles)
VSH = NT * 128 * (H * 65)     # V shard elems (vaug layout: per head v|keep)
ASH = VSH + KSH               # first AG shard: V(+ones) then K half 1
SM_SH = 2048                  # small AG shard (xn_pool 1024 + xc1p 1024) bf16
AT_SH = H * 65                # attn-partial AG shard (f32)
RG = [[0, 1, 2, 3], [4, 5, 6, 7]]
EPS = 1e-5


def _bf(a):
    return np.ascontiguousarray(np.asarray(a, np.float32)).astype(BF16)


def _f32(a):
    return np.ascontiguousarray(np.asarray(a, np.float32))


def _chunkR(wT):
    """[D, OUT] -> same-size array where chunk e ([128 out] x [D in]) is
    contiguous: R[e, p, d, j] = wT[d*128+p, e*128+j]."""
    Dd, OUT = wT.shape
    r = wT.reshape(Dd // 128, 128, OUT // 128, 128).transpose(2, 1, 0, 3)
    return np.ascontiguousarray(r).reshape(Dd, OUT)


def _f8(a):
    return np.ascontiguousarray(np.asarray(a, np.float32)).astype(
        ml_dtypes.float8_e4m3)


def build_program():
    import concourse.bass as bass
    import concourse.mybir as mybir
    import concourse.tile as tile

    f32 = mybir.dt.float32
    bf16 = mybir.dt.bfloat16

    nc = bass.Bass(num_devices=8)

    def din(name, shape, dt=bf16):
        return nc.declare_dram_parameter(name, shape, dt, isOutput=False)

    P = {}
    P["x_sh"] = din("x_sh", [TLOC, D], f32)
    P["ident"] = din("ident", [128, 128])
    P["cos_t"] = din("cos_t", [128, TLOC], f32)
    P["sinm_t"] = din("sinm_t", [128, TLOC], f32)
    P["keep_loc"] = din("keep_loc", [128, NT], f32)
    P["keep_lb"] = din("keep_lb", [128, NT])   # bf16 copy of keep_loc
    P["keep_f8"] = din("keep_f8", [128, NT], mybir.dt.float8e4)
    P["keep_all"] = din("keep_all", [128, KT])  # bf16, whole group
    for l in range(NL):
        for w in ("wq", "wk", "wv"):
            P[f"{w}T{l}"] = din(f"{w}T{l}", [D, D])
        P[f"bk{l}"] = din(f"bk{l}", [128, DT], f32)
        P[f"bv{l}"] = din(f"bv{l}", [1, D])
    P["bq0"] = din("bq0", [128, DT], f32)
    P["woT0"] = din("woT0", [D, D])
    P["w1T0"] = din("w1T0", [D, DFF])
    P["b1T0"] = din("b1T0", [128, FT], f32)
    P["w2T0"] = din("w2T0", [DFF, D])
    P["b20"] = din("b20", [1, D])
    P["bq1row"] = din("bq1row", [1, D], f32)
    P["wo1T"] = din("wo1T", [D, D])
    P["w1T1s"] = din("w1T1s", [D, D])
    P["b1row"] = din("b1row", [1, D], f32)
    P["w2T1s"] = din("w2T1s", [D, D])
    P["b21row"] = din("b21row", [1, D], f32)
    P["projTs"] = din("projTs", [D, 256])
    P["pbias"] = din("pbias", [1, 256], f32)
    P["out"] = nc.declare_dram_parameter("out", [1, 256], f32, isOutput=True)

    with tile.TileContext(nc) as tc:
        with contextlib.ExitStack() as es:
            _emit(nc, tc, es, P)
    _split_multiwaits(nc, mybir)
    return nc


def _split_multiwaits(nc, mybir):
    """Walrus caps sync commands on real compute ops; NoOps can hold many.
    Move multi-wait lists onto a NoOp inserted just before the instruction."""
    n = [0]

    def fresh():
        n[0] += 1
        return f"I-syncsplit-{n[0]}"

    for fn in nc.m.functions:
        for blk in fn.blocks:
            out = []
            for inst in blk.instructions:
                si = inst.sync_info
                if (si is not None and si.on_wait and len(si.on_wait) > 1
                        and type(inst).__name__ != "InstNoOp"):
                    for w in list(si.on_wait):
                        out.append(mybir.InstNoOp(
                            name=fresh(), ins=[], outs=[], engine=inst.engine,
                            sync_info=mybir.SyncInfo(on_wait=[w], on_update=[]),
                            bass_nofuse=True))
                    inst.sync_info = mybir.SyncInfo(
                        on_wait=[], on_update=list(si.on_update))
                out.append(inst)
            blk.instructions = out


def _emit(nc, tc, es, P):
    import concourse.bass as bass
    import concourse.mybir as mybir

    f32 = mybir.dt.float32
    bf16 = mybir.dt.bfloat16
    f8 = mybir.dt.float8e4
    AF = mybir.ActivationFunctionType
    OP = mybir.AluOpType
    AX = mybir.AxisListType
    ts = bass.ts
    ec = es.enter_context

    const = ec(tc.tile_pool(name="const", bufs=1))
    persist = ec(tc.tile_pool(name="persist", bufs=1))
    act = ec(tc.tile_pool(name="act", bufs=2))
    wbuf = ec(tc.tile_pool(name="wbuf", bufs=2))
    wpers = ec(tc.tile_pool(name="wpers", bufs=1))  # 8x [128,D] bf16, reused
    rope_p = ec(tc.tile_pool(name="rope", bufs=2))
    ptp = ec(tc.tile_pool(name="ptp", bufs=3))
    small = ec(tc.tile_pool(name="small", bufs=1))
    stats = ec(tc.tile_pool(name="stats", bufs=2))
    dram = ec(tc.tile_pool(name="dram", bufs=1, space="DRAM"))

    dma = nc.sync.dma_start

    def fview(dram_tile, off, p, f):
        a = dram_tile[:]
        return bass.AP(tensor=a.tensor, offset=a.offset + off,
                       ap=[[f, p], [1, f]])

    # ---------------- constants ----------------
    ident_sb = const.tile([128, 128], bf16, tag="ident", name="ident")
    dma(out=ident_sb[:], in_=P["ident"][:])
    cos_sb = const.tile([128, TLOC], f32, tag="cos", name="cos")
    dma(out=cos_sb[:], in_=P["cos_t"][:])
    sinm_sb = const.tile([128, TLOC], f32, tag="sinm", name="sinm")
    dma(out=sinm_sb[:], in_=P["sinm_t"][:])
    keep_sb = const.tile([128, NT], f32, tag="keep", name="keep")
    dma(out=keep_sb[:], in_=P["keep_loc"][:])
    eps_sb = const.tile([128, 1], f32, tag="eps", name="eps")
    nc.vector.memset(eps_sb[:], EPS)
    bq0_sb = const.tile([128, DT], f32, tag="bq0", name="bq0")
    dma(out=bq0_sb[:], in_=P["bq0"][:])
    bk_sb, bv_bc = {}, {}
    for l in range(NL):
        bk_sb[l] = const.tile([128, DT], f32, tag=f"bk{l}", name=f"bk{l}")
        dma(out=bk_sb[l][:], in_=P[f"bk{l}"][:])
        bv_bc[l] = const.tile([128, D], bf16, tag=f"bvbc{l}", name=f"bvbc{l}")
        dma(out=bv_bc[l][:], in_=P[f"bv{l}"][:].to_broadcast([128, D]))
    b1T0_sb = const.tile([128, FT], f32, tag="b1T0", name="b1T0")
    dma(out=b1T0_sb[:], in_=P["b1T0"][:])
    b2bc = const.tile([128, D], bf16, tag="b2bc", name="b2bc")
    dma(out=b2bc[:], in_=P["b20"][:].to_broadcast([128, D]))
    pbias_sb = const.tile([1, 256], f32, tag="pbias", name="pbias")
    dma(out=pbias_sb[:], in_=P["pbias"][:])
    bq1r_sb = const.tile([1, D], f32, tag="bq1r", name="bq1r")
    dma(out=bq1r_sb[:], in_=P["bq1row"][:])
    b1r_sb = const.tile([1, D], f32, tag="b1r", name="b1r")
    dma(out=b1r_sb[:], in_=P["b1row"][:])
    b21r_sb = const.tile([1, D], f32, tag="b21r", name="b21r")
    dma(out=b21r_sb[:], in_=P["b21row"][:])

    # residual stream f32, token-major
    x_res = [persist.tile([128, D], f32, tag=f"xres{t}", name=f"xres{t}")
             for t in range(NT)]
    for t in range(NT):
        dma(out=x_res[t][:], in_=P["x_sh"][ts(t, 128), :])

    # ---------------- helpers ----------------
    def ln_to_xnT(tags, psp):
        """LN of x_res (no affine; folded into weights) -> DT x [128,TLOC]
        bf16, feature-major (transposed)."""
        xnT = [persist.tile([128, TLOC], bf16, tag=tags[d], name=f"{tags[d]}_n")
               for d in range(DT)]
        for t in range(NT):
            st = stats.tile([128, 2, 6], f32, tag="bnst", name="bnst")
            nc.vector.bn_stats(out=st[:, 0, :], in_=x_res[t][:, 0:512])
            nc.vector.bn_stats(out=st[:, 1, :], in_=x_res[t][:, 512:1024])
            mv = stats.tile([128, 2], f32, tag="bnmv", name="bnmv")
            nc.vector.bn_aggr(out=mv[:], in_=st[:])
            std = stats.tile([128, 1], f32, tag="std", name="std")
            nc.scalar.activation(out=std[:], in_=mv[:, 1:2], func=AF.Sqrt,
                                 bias=eps_sb[:], scale=1.0)
            r = stats.tile([128, 1], f32, tag="rstd", name="rstd")
            nc.vector.reciprocal(out=r[:], in_=std[:])
            xsub = act.tile([128, D], f32, tag="xsub", name="xsub")
            nc.vector.tensor_scalar_sub(out=xsub[:], in0=x_res[t][:],
                                        scalar1=mv[:, 0:1])
            xn = act.tile([128, D], bf16, tag="xn", name="xn")
            nc.scalar.activation(out=xn[:], in_=xsub[:], func=AF.Identity,
                                 bias=0.0, scale=r[:])
            for d in range(DT):
                pt = psp.tile([128, 128], bf16, tag="tp", name="tp")
                nc.tensor.transpose(pt[:], xn[:, ts(d, 128)], ident_sb[:])
                nc.scalar.activation(out=xnT[d][:, ts(t, 128)], in_=pt[:],
                                     func=AF.Copy)
        return xnT

    def qk_proj(xnT, w_dram, bias_sb, tags, psp, rng=None, dest=None,
                odt=bf16):
        oT = dest if dest is not None else {}
        wa = w_dram[:]
        for e in (rng if rng is not None else range(DT)):
            if e not in oT:
                oT[e] = persist.tile([128, TLOC], odt, tag=tags[e],
                                     name=f"{tags[e]}_p")
            wc = wbuf.tile([128, DT, 128], bf16, tag="wchunk", name="wchunk")
            src = bass.AP(tensor=wa.tensor,
                          offset=wa.offset + e * 128 * DT * 128,
                          ap=[[DT * 128, 128], [128, DT], [1, 128]])
            dma(out=wc[:], in_=src)
            pq = psp.tile([128, 512], f32, tag="pq", name="pq")
            for d in range(DT):
                nc.tensor.matmul(pq[:, 0:TLOC], wc[:, d, :], xnT[d][:],
                                 start=(d == 0), stop=(d == DT - 1))
            nc.scalar.activation(out=oT[e][:], in_=pq[:, 0:TLOC],
                                 func=AF.Identity,
                                 bias=bias_sb[:, e:e + 1], scale=1.0)
        return oT

    def v_proj(xnT, w_dram, bvbc, tags, psh):
        """-> NT tiles [128, H*65] bf16 in vaug layout (per head v(64)|keep),
        bias added, pad rows zeroed, keep column from keep_lb."""
        v = [persist.tile([128, H * 65], bf16, tag=tags[t],
                          name=f"{tags[t]}_v") for t in range(NT)]
        for hlf in range(2):
            held = [psh.tile([128, 512], f32, tag=f"vh{i}", name=f"vh{i}")
                    for i in range(NT)]
            for d in range(DT):
                wvc = wbuf.tile([128, 512], bf16, tag="wvchunk", name="wvchunk")
                dma(out=wvc[:], in_=w_dram[ts(d, 128), ts(hlf, 512)])
                for t in range(NT):
                    nc.tensor.matmul(held[t][:], xnT[d][:, ts(t, 128)], wvc[:],
                                     start=(d == 0), stop=(d == DT - 1),
                                     skip_group_check=True)
            for t in range(NT):
                a = v[t][:]
                vout = bass.AP(tensor=a.tensor,
                               offset=a.offset + hlf * 8 * 65,
                               ap=[a.ap[0], [65, 8], [1, 64]])
                nc.vector.tensor_tensor(out=vout, in0=held[t][:],
                                        in1=bvbc[:, ts(hlf, 512)], op=OP.add)
                nc.vector.tensor_scalar_mul(out=vout, in0=vout,
                                            scalar1=keep_sb[:, t:t + 1])
        for t in range(NT):
            a = v[t][:]
            ones_dst = bass.AP(tensor=a.tensor, offset=a.offset + 64,
                               ap=[a.ap[0], [65, H]])
            kb = P["keep_lb"][:, t:t + 1]
            dma(out=ones_dst, in_=bass.AP(tensor=kb.tensor, offset=kb.offset,
                                          ap=[kb.ap[0], [0, H]]))
        return v

    def rope(q):
        shuf = rope_p.tile([128, TLOC], bf16, tag="shuf", name="shuf")
        for blk in range(2):
            b0 = 64 * blk
            nc.vector.tensor_copy(out=shuf[b0:b0 + 32, :],
                                  in_=q[b0 + 32:b0 + 64, :])
            nc.vector.tensor_copy(out=shuf[b0 + 32:b0 + 64, :],
                                  in_=q[b0:b0 + 32, :])
        qc = rope_p.tile([128, TLOC], bf16, tag="qcos", name="qcos")
        nc.vector.tensor_tensor(out=qc[:], in0=q[:], in1=cos_sb[:], op=OP.mult)
        qs = rope_p.tile([128, TLOC], bf16, tag="qsin", name="qsin")
        nc.vector.tensor_tensor(out=qs[:], in0=shuf[:], in1=sinm_sb[:],
                                op=OP.mult)
        nc.vector.tensor_tensor(out=q[:], in0=qc[:], in1=qs[:], op=OP.add)

    # ================= LAYER 1 =================
    xnT_tags = [f"xnT{d}" for d in range(DT)]
    qT_tags = [f"qT{d}" for d in range(DT)]
    kT_tags = [f"kT{d}" for d in range(DT)]
    vg_tags = [f"vg{t}" for t in range(NT)]

    with tc.tile_pool(name="ps_a", bufs=2, space="PSUM") as ps_a, \
         tc.tile_pool(name="ps_b", bufs=1, space="PSUM") as ps_b:
        xnT = ln_to_xnT(xnT_tags, ps_a)

        # V (vaug layout) + K half 1 in one AG, K half 2 in a second.
        v_sb = v_proj(xnT, P["wvT0"], bv_bc[0], vg_tags, ps_b)
        ag_a_in = dram.tile([ASH], bf16, tag="agai1", name="agai1")
        ag_a_out = dram.tile([G * ASH], bf16, tag="agao1", name="agao1")
        for t in range(NT):
            dma(out=fview(ag_a_in, t * 128 * H * 65, 128, H * 65),
                in_=v_sb[t][:])
        kT = {}
        qk_proj(xnT, P["wkT0"], bk_sb[0], kT_tags, ps_a, range(4), dest=kT)
        for e in range(4):
            rope(kT[e])
            dma(out=fview(ag_a_in, VSH + e * 128 * TLOC, 128, TLOC),
                in_=kT[e][:])
        nc.gpsimd.collective_compute(
            "AllGather", OP.bypass, replica_groups=RG,
            ins=[ag_a_in[:]], outs=[ag_a_out[:]])

        ag_b_in = dram.tile([KSH], bf16, tag="agbi", name="agbi")
        ag_b_out = dram.tile([G * KSH], bf16, tag="agbo", name="agbo")
        qk_proj(xnT, P["wkT0"], bk_sb[0], kT_tags, ps_a, range(4, 8), dest=kT)
        for e in range(4, 8):
            rope(kT[e])
            dma(out=fview(ag_b_in, (e - 4) * 128 * TLOC, 128, TLOC),
                in_=kT[e][:])
        nc.gpsimd.collective_compute(
            "AllGather", OP.bypass, replica_groups=RG,
            ins=[ag_b_in[:]], outs=[ag_b_out[:]])

        qT = qk_proj(xnT, P["wqT0"], bq0_sb, qT_tags, ps_a)
        for e in range(DT):
            rope(qT[e])

        # khd: [128, CAP] bf16 per feature tile; one 3D DMA each
        khd = []
        for dt_ in range(DT):
            t_ = persist.tile([128, CAP], bf16, tag=f"khd{dt_}",
                              name=f"khd{dt_}")
            khd.append(t_)
            if dt_ < 4:
                ao = ag_a_out[:]
                src = bass.AP(
                    tensor=ao.tensor,
                    offset=ao.offset + VSH + dt_ * 128 * TLOC,
                    ap=[[TLOC, 128], [ASH, G], [1, TLOC]])
            else:
                ao = ag_b_out[:]
                src = bass.AP(
                    tensor=ao.tensor,
                    offset=ao.offset + (dt_ - 4) * 128 * TLOC,
                    ap=[[TLOC, 128], [KSH, G], [1, TLOC]])
            dst = bass.AP(tensor=t_[:].tensor, offset=t_[:].offset,
                          ap=[t_[:].ap[0], [TLOC, G], [1, TLOC]])
            dma(out=dst, in_=src)

        # vaug[k]: contiguous copies from the gathered vaug-layout V
        vaug = []
        for k in range(KT):
            va = persist.tile([128, H * 65], bf16, tag=f"vg_a{k}",
                              name=f"vg_a{k}")
            vaug.append(va)
            r, lt = k // NT, k % NT
            dma(out=va[:],
                in_=fview(ag_a_out, r * ASH + lt * 128 * H * 65,
                          128, H * 65))

    # attention: scores transposed [keys, queries], head pairs row-packed
    oT = [persist.tile([128, TLOC], bf16, tag=xnT_tags[d], name=f"oT{d}")
          for d in range(DT)]
    # dens live 4-per-tile at partition starts {0,32,64,96} (engine ops
    # require 32-aligned start partitions)
    den_sb = [small.tile([128, TLOC], f32, tag=f"den{i}", name=f"den{i}")
              for i in range(4)]
    for i in range(4):
        nc.vector.memset(den_sb[i][:], 1.0)
    with tc.tile_pool(name="ps_sc", bufs=2, space="PSUM") as ps_sc, \
         tc.tile_pool(name="ps_pav", bufs=2, space="PSUM") as ps_pav:
        for dt_ in range(DT):
            pav = [ps_pav.tile([65, 512], f32, tag=f"pav{hh}",
                               name=f"pav{hh}_{dt_}") for hh in range(2)]
            pts = {}

            def _av(k):
                for hh in range(2):
                    h = 2 * dt_ + hh
                    nc.tensor.matmul(pav[hh][0:65, 0:TLOC],
                                     vaug[k][:, 65 * h:65 * h + 65],
                                     pts[k][:, hh, :],
                                     start=(k == 0), stop=(k == KT - 1),
                                     skip_group_check=True)

            for k in range(KT):
                sc = ps_sc.tile([128, 2, 512], f32, tag="sc", name="sc")
                for hh in range(2):
                    off = 64 * hh
                    nc.tensor.matmul(sc[:, hh, 0:TLOC],
                                     khd[dt_][off:off + 64, ts(k, 128)],
                                     qT[dt_][off:off + 64, :],
                                     start=True, stop=True,
                                     skip_group_check=True)
                pt = ptp.tile([128, 2, TLOC], bf16, tag="pt", name="pt")
                nc.scalar.activation(out=pt[:], in_=sc[:, :, 0:TLOC],
                                     func=AF.Exp, bias=0.0, scale=0.125)
                pts[k] = pt
                # pipeline: AV trails scores by one k so the in-order PE
                # queue never waits on the exp of the current tile
                if k >= 1:
                    _av(k - 1)
            _av(KT - 1)
            for hh in range(2):
                h = 2 * dt_ + hh
                off = 64 * hh
                nc.vector.tensor_copy(out=oT[dt_][off:off + 64, :],
                                      in_=pav[hh][0:64, 0:TLOC])
                dp = 32 * (h % 4)
                nc.vector.tensor_copy(
                    out=den_sb[h // 4][dp:dp + 1, :],
                    in_=pav[hh][64:65, 0:TLOC])

    invd_d = dram.tile([H * TLOC], f32, tag="invd_d", name="invd_d")
    for i in range(4):
        inv4 = small.tile([128, TLOC], f32, tag="inv4", name=f"inv4_{i}")
        nc.vector.reciprocal(out=inv4[:], in_=den_sb[i][:])
        a = inv4[:]
        src = bass.AP(tensor=a.tensor, offset=a.offset,
                      ap=[[a.ap[0][0] * 32, 4], [1, TLOC]])
        dma(out=fview(invd_d, 4 * i * TLOC, 4, TLOC), in_=src)
    for dt_ in range(DT):
        bc = act.tile([128, TLOC], f32, tag="invdbc", name="invdbc")
        ia = invd_d[:]
        src = bass.AP(tensor=ia.tensor, offset=ia.offset + 2 * dt_ * TLOC,
                      ap=[[TLOC, 2], [0, 64], [1, TLOC]])
        dma(out=bc[:], in_=src)
        nc.vector.tensor_tensor(out=oT[dt_][:], in0=oT[dt_][:], in1=bc[:],
                                op=OP.mult)

    # O-projection + residual
    w8 = [wpers.tile([128, D], bf16, tag=f"w8_{d}", name=f"wo8_{d}")
          for d in range(DT)]
    for d in range(DT):
        dma(out=w8[d][:], in_=P["woT0"][ts(d, 128), :])
    with tc.tile_pool(name="ps_c", bufs=2, space="PSUM") as ps_c:
        for t in range(NT):
            for hlf in range(2):
                po = ps_c.tile([128, 512], f32, tag="po", name="po")
                for d in range(DT):
                    nc.tensor.matmul(po[:], oT[d][:, ts(t, 128)],
                                     w8[d][:, ts(hlf, 512)],
                                     start=(d == 0), stop=(d == DT - 1))
                nc.vector.tensor_tensor(out=x_res[t][:, ts(hlf, 512)],
                                        in0=x_res[t][:, ts(hlf, 512)],
                                        in1=po[:], op=OP.add)

    # tail-weight prefetch into now-dead SBUF: wq1 -> w8 (during MLP1),
    # w1T1s -> khd slots, w2T1s -> vg_a slots, proj -> its own tiles
    for d in range(DT):
        dma(out=w8[d][:], in_=P["wqT1"][ts(d, 128), :])
    w1s_sb = [persist.tile([128, 512], bf16, tag=f"khd{d}", name=f"w1s{d}")
              for d in range(DT)]
    w1s_sb += [persist.tile([128, 512], bf16, tag=f"vg_a{k}",
                            name=f"w1sb{k}") for k in range(3, 11)]
    for d in range(DT):
        dma(out=w1s_sb[d][:], in_=P["w1T1s"][ts(d, 128), 0:512])
        dma(out=w1s_sb[8 + d][:], in_=P["w1T1s"][ts(d, 128), 512:1024])

    proj_sb = [wbuf.tile([128, 256], bf16, tag=f"projc{d}", name=f"projc{d}")
               for d in range(DT)]
    for d in range(DT):
        dma(out=proj_sb[d][:], in_=P["projTs"][ts(d, 128), :])

    with tc.tile_pool(name="ps_d", bufs=2, space="PSUM") as ps_d:
        xn2T = ln_to_xnT(qT_tags, ps_d)  # reuse qT slots (dead)
    for t in range(NT):  # pre-add the MLP output bias while PE runs MLP
        nc.vector.tensor_tensor(out=x_res[t][:], in0=x_res[t][:],
                                in1=b2bc[:], op=OP.add)

    with tc.tile_pool(name="ps_mlp", bufs=2, space="PSUM") as ps_mlp, \
         tc.tile_pool(name="ps_hld", bufs=1, space="PSUM") as ps_hld:
        held = [ps_hld.tile([128, 2, 512], f32, tag=f"mh{t}", name=f"mh{t}")
                for t in range(NT)]
        hTs, w2cs = {}, {}

        def _mlp2nd(f):
            for t in range(NT):
                for hlf in range(2):
                    nc.tensor.matmul(held[t][:, hlf, :], hTs[f][:, ts(t, 128)],
                                     w2cs[f][:, ts(hlf, 512)],
                                     start=(f == 0), stop=(f == FT - 1),
                                     skip_group_check=True)

        for f in range(FT):
            wc = wbuf.tile([128, DT, 128], bf16, tag="w1chunk", name="w1chunk")
            wa = P["w1T0"][:]
            src = bass.AP(tensor=wa.tensor,
                          offset=wa.offset + f * 128 * DT * 128,
                          ap=[[DT * 128, 128], [128, DT], [1, 128]])
            dma(out=wc[:], in_=src)
            ph = ps_mlp.tile([128, 512], f32, tag="ph", name="ph")
            for d in range(DT):
                nc.tensor.matmul(ph[:, 0:TLOC], wc[:, d, :], xn2T[d][:],
                                 start=(d == 0), stop=(d == DT - 1))
            hT = act.tile([128, TLOC], bf16, tag="hT", name="hT")
            nc.scalar.activation(out=hT[:], in_=ph[:, 0:TLOC], func=AF.Gelu,
                                 bias=b1T0_sb[:, f:f + 1], scale=1.0)
            hTs[f] = hT
            w2c = wbuf.tile([128, D], bf16, tag="w2chunk", name="w2chunk")
            dma(out=w2c[:], in_=P["w2T0"][ts(f, 128), :])
            w2cs[f] = w2c
            if f >= 1:
                _mlp2nd(f - 1)
        _mlp2nd(FT - 1)
        for t in range(NT):
            for hlf in range(2):
                nc.vector.tensor_tensor(out=x_res[t][:, ts(hlf, 512)],
                                        in0=x_res[t][:, ts(hlf, 512)],
                                        in1=held[t][:, hlf, :], op=OP.add)

    # ================= LAYER 2 =================
    with tc.tile_pool(name="ps_e", bufs=2, space="PSUM") as ps_e:
        xn3T = ln_to_xnT(xnT_tags, ps_e)

        # small AG: pool xn (feature-major col 0) + pool residual row, bf16
        xc1p_bf = small.tile([1, D], bf16, tag="rb_a", name="xc1pbf")
        nc.scalar.activation(out=xc1p_bf[:], in_=x_res[0][0:1, :], func=AF.Copy)
        ag_s_in = dram.tile([SM_SH], bf16, tag="agsi", name="agsi")
        ag_s_out = dram.tile([G * SM_SH], bf16, tag="agso", name="agso")
        for d in range(DT):
            dma(out=fview(ag_s_in, d * 128, 128, 1), in_=xn3T[d][:, 0:1])
        dma(out=fview(ag_s_in, D, 1, D), in_=xc1p_bf[:])
        nc.gpsimd.collective_compute(
            "AllGather", OP.bypass, replica_groups=RG,
            ins=[ag_s_in[:]], outs=[ag_s_out[:]])

        kT2 = qk_proj(xn3T, P["wkT1"], bk_sb[1], kT_tags, ps_e)
        for e in range(DT):
            rope(kT2[e])

    with tc.tile_pool(name="ps_f", bufs=1, space="PSUM") as ps_f:
        v2 = v_proj(xn3T, P["wvT1"], bv_bc[1], vg_tags, ps_f)

    vaug2 = v2  # v_proj already emits the vaug layout with keep column

    with tc.tile_pool(name="ps_g", bufs=1, space="PSUM") as ps:
        # read back pool xn + residual from shard 0 of small AG
        xnp = small.tile([128, DT], bf16, tag="xnp", name="xnp")
        ao = ag_s_out[:]
        dma(out=xnp[:], in_=bass.AP(tensor=ao.tensor, offset=ao.offset,
                                    ap=[[1, 128], [128, DT]]))
        xc1p = small.tile([1, D], f32, tag="rf_a", name="xc1p")
        xc1p_b2 = small.tile([1, D], bf16, tag="rb_b", name="xc1pb2")
        dma(out=xc1p_b2[:], in_=bass.AP(tensor=ao.tensor,
                                        offset=ao.offset + D,
                                        ap=[[1, 1], [1, D]]))
        nc.vector.tensor_copy(out=xc1p[:], in_=xc1p_b2[:])

        # q2 row = xnp.T @ Wq1T + bias (w8 holds prefetched Wq1)
        rps = ps.tile([1, D], f32, tag="rps", name="q2ps")
        for hlf in range(2):
            for d in range(DT):
                nc.tensor.matmul(rps[:, ts(hlf, 512)], xnp[:, d:d + 1],
                                 w8[d][:, ts(hlf, 512)],
                                 start=(d == 0), stop=(d == DT - 1),
                                 skip_group_check=True)
        for d in range(DT):  # wo1 -> w8; overlaps partial attention
            dma(out=w8[d][:], in_=P["wo1T"][ts(d, 128), :])
        q2row = small.tile([1, D], bf16, tag="rb_c", name="q2row")
        q2f = small.tile([1, D], f32, tag="rf_b", name="q2f")
        nc.vector.tensor_tensor(out=q2f[:], in0=rps[:], in1=bq1r_sb[:],
                                op=OP.add)
        nc.vector.tensor_copy(out=q2row[:], in_=q2f[:])
        q2_d = dram.tile([D], bf16, tag="q2d", name="q2d")
        dma(out=fview(q2_d, 0, 1, D), in_=q2row[:])
        q2T = small.tile([128, DT], bf16, tag="q2T", name="q2T")
        qd = q2_d[:]
        dma(out=q2T[:], in_=bass.AP(tensor=qd.tensor, offset=qd.offset,
                                    ap=[[1, 128], [128, DT]]))

        # partial pool attention over local keys
        p2ps = ps.tile([128, 2 * NT * DT], f32, tag="p2ps", name="p2ps")
        for dt_ in range(DT):
            for k in range(NT):
                for hh in range(2):
                    off = 64 * hh
                    c = dt_ * 2 * NT + hh * NT + k
                    nc.tensor.matmul(p2ps[:, c:c + 1],
                                     kT2[dt_][off:off + 64, ts(k, 128)],
                                     q2T[off:off + 64, dt_:dt_ + 1],
                                     start=True, stop=True,
                                     skip_group_check=True)
        p2sb = small.tile([128, 2 * NT * DT], bf16, tag="p2sb", name="p2sb")
        nc.scalar.activation(out=p2sb[:], in_=p2ps[:], func=AF.Exp,
                             bias=0.0, scale=0.125)
        # h outer / k inner: accumulation groups must be sequential within a
        # PSUM bank (start=True clears the whole bank's has_written bits)
        o2ps = ps.tile([65, 16], f32, tag="o2ps", name="o2ps")
        for h in range(H):
            for k in range(NT):
                c = (h // 2) * 2 * NT + (h % 2) * NT + k
                nc.tensor.matmul(o2ps[:, h:h + 1],
                                 vaug2[k][:, 65 * h:65 * h + 65],
                                 p2sb[:, c:c + 1],
                                 start=(k == 0), stop=(k == NT - 1),
                                 skip_group_check=True)
        # partials -> AG -> sum
        part_sb = small.tile([65, 16], f32, tag="part", name="part")
        nc.vector.tensor_copy(out=part_sb[:], in_=o2ps[:])
        ag_a_in = dram.tile([AT_SH], f32, tag="agai", name="agai")
        ag_a_out = dram.tile([G * AT_SH], f32, tag="agao", name="agao")
        dma(out=fview(ag_a_in, 0, 65, 16), in_=part_sb[:])
        nc.gpsimd.collective_compute(
            "AllGather", OP.bypass, replica_groups=RG,
            ins=[ag_a_in[:]], outs=[ag_a_out[:]])
        sums = small.tile([65, G, 16], f32, tag="sums", name="sums")
        for r in range(G):
            dma(out=sums[:, r, :], in_=fview(ag_a_out, r * AT_SH, 65, 16))
        tot = small.tile([65, 16], f32, tag="tot", name="tot")
        nc.vector.tensor_tensor(out=sums[:, 0, :], in0=sums[:, 0, :],
                                in1=sums[:, 1, :], op=OP.add)
        nc.vector.tensor_tensor(out=sums[:, 2, :], in0=sums[:, 2, :],
                                in1=sums[:, 3, :], op=OP.add)
        nc.vector.tensor_tensor(out=tot[:], in0=sums[:, 0, :],
                                in1=sums[:, 2, :], op=OP.add)
        # o2 feature-major [128, DT] f32 via 2 DMAs; divide -> bf16
        o2f = small.tile([128, DT], f32, tag="o2f", name="o2f")
        ta = tot[:]
        for a in range(2):
            dma(out=o2f[64 * a:64 * a + 64, :],
                in_=bass.AP(tensor=ta.tensor, offset=ta.offset + a,
                            ap=[[ta.ap[0][0], 64], [2, DT]]))
        den2 = small.tile([1, H], f32, tag="den2", name="den2")
        nc.vector.tensor_copy(out=den2[:], in_=tot[64:65, :])
        invd2 = small.tile([1, H], f32, tag="invd2", name="invd2")
        nc.vector.reciprocal(out=invd2[:], in_=den2[:])
        den_d = dram.tile([H], f32, tag="den_d", name="den_d")
        dma(out=fview(den_d, 0, 1, H), in_=invd2[:])
        o2bc = small.tile([128, DT], f32, tag="o2bc", name="o2bc")
        dd = den_d[:]
        for a in range(2):
            dma(out=o2bc[64 * a:64 * a + 64, :],
                in_=bass.AP(tensor=dd.tensor, offset=dd.offset + a,
                            ap=[[0, 64], [2, DT]]))
        o2sb = small.tile([128, DT], bf16, tag="o2sb", name="o2sb")
        nc.vector.tensor_tensor(out=o2sb[:], in0=o2f[:], in1=o2bc[:],
                                op=OP.mult)

        # x2 row = xc1p + o2 @ Wo2 (w8 holds prefetched Wo1)
        rps2 = ps.tile([1, D], f32, tag="rps", name="x2ps")
        for hlf in range(2):
            for d in range(DT):
                nc.tensor.matmul(rps2[:, ts(hlf, 512)], o2sb[:, d:d + 1],
                                 w8[d][:, ts(hlf, 512)],
                                 start=(d == 0), stop=(d == DT - 1),
                                 skip_group_check=True)
        for d in range(DT):  # w2T1s -> w8; overlaps rowLN/gelu below
            dma(out=w8[d][:], in_=P["w2T1s"][ts(d, 128), :])
        x2row = small.tile([1, D], f32, tag="rf_c", name="x2row")
        nc.vector.tensor_tensor(out=x2row[:], in0=xc1p[:], in1=rps2[:],
                                op=OP.add)

        def row_ln(xrow, out_tag, nm):
            """LN of a [1, D] f32 row -> [1, D] bf16 (no affine)."""
            s1 = small.tile([1, 1], f32, tag="lns1", name=f"{nm}s1")
            nc.vector.reduce_sum(out=s1[:], in_=xrow[:], axis=AX.X)
            mean = small.tile([1, 1], f32, tag="lnmean", name=f"{nm}mean")
            nc.vector.tensor_scalar_mul(out=mean[:], in0=s1[:],
                                        scalar1=1.0 / D)
            xc = small.tile([1, D], f32, tag="rf_d", name=f"{nm}xc")
            nc.vector.tensor_scalar_sub(out=xc[:], in0=xrow[:],
                                        scalar1=mean[:])
            sq = small.tile([1, D], f32, tag="rf_e", name=f"{nm}sq")
            nc.vector.tensor_tensor(out=sq[:], in0=xc[:], in1=xc[:],
                                    op=OP.mult)
            s2 = small.tile([1, 1], f32, tag="lns2", name=f"{nm}s2")
            nc.vector.reduce_sum(out=s2[:], in_=sq[:], axis=AX.X)
            std = small.tile([1, 1], f32, tag="lnstd", name=f"{nm}sd")
            nc.scalar.activation(out=std[:], in_=s2[:], func=AF.Sqrt,
                                 bias=eps_sb[0:1, :], scale=1.0 / D)
            rr = small.tile([1, 1], f32, tag="lnrr", name=f"{nm}rr")
            nc.vector.reciprocal(out=rr[:], in_=std[:])
            xo = small.tile([1, D], bf16, tag=out_tag, name=f"{nm}o")
            nc.scalar.activation(out=xo[:], in_=xc[:], func=AF.Identity,
                                 bias=0.0, scale=rr[:])
            return xo

        def row_to_fmaj(row_bf, tag, nm):
            """[1, D] bf16 row -> [128, DT] bf16 feature-major via DRAM."""
            rd = dram.tile([D], bf16, tag=f"{tag}_d", name=f"{nm}_d")
            dma(out=fview(rd, 0, 1, D), in_=row_bf[:])
            fm = small.tile([128, DT], bf16, tag=tag, name=nm)
            a = rd[:]
            dma(out=fm[:], in_=bass.AP(tensor=a.tensor, offset=a.offset,
                                       ap=[[1, 128], [128, DT]]))
            return fm

        xn2f = row_ln(x2row, "rb_d", "ln2f")
        xn2fm = row_to_fmaj(xn2f, "fm_a", "xn2fm")

        # sharded MLP2 (this core's 1024 DFF rows), weights prefetched
        hps = ps.tile([1, D], f32, tag="rps", name="hps")
        for hlf in range(2):
            for d in range(DT):
                nc.tensor.matmul(hps[:, ts(hlf, 512)], xn2fm[:, d:d + 1],
                                 w1s_sb[8 * hlf + d][:],
                                 start=(d == 0), stop=(d == DT - 1),
                                 skip_group_check=True)
        hrow_f = small.tile([1, D], f32, tag="rf_b", name="hrowf")
        nc.vector.tensor_tensor(out=hrow_f[:], in0=hps[:], in1=b1r_sb[:],
                                op=OP.add)
        hrow = small.tile([1, D], bf16, tag="rb_c", name="hrow")
        nc.scalar.activation(out=hrow[:], in_=hrow_f[:], func=AF.Gelu,
                             bias=0.0, scale=1.0)
        hfm = row_to_fmaj(hrow, "fm_b", "hfm")
        yps = ps.tile([1, D], f32, tag="rps", name="yps")
        for hlf in range(2):
            for d in range(DT):
                nc.tensor.matmul(yps[:, ts(hlf, 512)], hfm[:, d:d + 1],
                                 w8[d][:, ts(hlf, 512)],
                                 start=(d == 0), stop=(d == DT - 1),
                                 skip_group_check=True)
        y2row = small.tile([1, D], f32, tag="rf_b", name="y2row")
        nc.vector.tensor_copy(out=y2row[:], in_=yps[:])
        ag_m_in = dram.tile([D], f32, tag="agmi", name="agmi")
        ag_m_out = dram.tile([G * D], f32, tag="agmo", name="agmo")
        dma(out=fview(ag_m_in, 0, 1, D), in_=y2row[:])
        nc.gpsimd.collective_compute(
            "AllGather", OP.bypass, replica_groups=RG,
            ins=[ag_m_in[:]], outs=[ag_m_out[:]])
        yacc = small.tile([1, D], f32, tag="rf_d", name="yacc")
        dma(out=yacc[:], in_=fview(ag_m_out, 0, 1, D))
        for r in range(1, G):
            ypart = small.tile([1, D], f32, tag="rf_e", name=f"ypart{r}")
            dma(out=ypart[:], in_=fview(ag_m_out, r * D, 1, D))
            nc.vector.tensor_tensor(out=yacc[:], in0=yacc[:], in1=ypart[:],
                                    op=OP.add)
        x3row = small.tile([1, D], f32, tag="rf_a", name="x3row")
        nc.vector.tensor_tensor(out=x3row[:], in0=x2row[:], in1=yacc[:],
                                op=OP.add)
        nc.vector.tensor_tensor(out=x3row[:], in0=x3row[:], in1=b21r_sb[:],
                                op=OP.add)

        xn3 = row_ln(x3row, "rb_d", "ln3")
        xn3fm = row_to_fmaj(xn3, "fm_a", "xn3fm")
        pps = ps.tile([1, 256], f32, tag="pps", name="pps")
        for d in range(DT):
            nc.tensor.matmul(pps[:], xn3fm[:, d:d + 1], proj_sb[d][:],
                             start=(d == 0), stop=(d == DT - 1),
                             skip_group_check=True)
        outsb = small.tile([1, 256], f32, tag="rf_e", name="outsb")
        nc.vector.tensor_tensor(out=outsb[:], in0=pps[:], in1=pbias_sb[:],
                                op=OP.add)
        dma(out=P["out"][:], in_=outsb[:])


def _host_prep(inputs):
    x = _f32(inputs["x"])
    mask = np.asarray(inputs["attention_mask"])
    pool = _f32(inputs["pool_token"]).reshape(1, D)

    Wq, Wk, Wv, Wo = (_f32(inputs[k]) for k in ("Wq", "Wk", "Wv", "Wo"))
    g1, b1l = _f32(inputs["ln1_g"]), _f32(inputs["ln1_b"])
    g2, b2l = _f32(inputs["ln2_g"]), _f32(inputs["ln2_b"])
    W1, b1 = _f32(inputs["W1"]), _f32(inputs["b1"])
    W2, b2 = _f32(inputs["W2"]), _f32(inputs["b2"])
    outg, outb = _f32(inputs["out_g"]), _f32(inputs["out_b"])
    pW, pb = _f32(inputs["proj_W"]), _f32(inputs["proj_b"])

    com = {"ident": _bf(np.eye(128))}
    for l in range(NL):
        wq_c = _chunkR((Wq[l] * g1[l][None, :]).T) if l == 0 else \
            (Wq[l] * g1[l][None, :]).T
        com[f"wqT{l}"] = _bf(wq_c)
        com[f"wkT{l}"] = _bf(_chunkR((Wk[l] * g1[l][None, :]).T))
        com[f"wvT{l}"] = _bf((Wv[l] * g1[l][None, :]).T)
        com[f"bk{l}"] = _f32((b1l[l] @ Wk[l].T).reshape(DT, 128).T)
        com[f"bv{l}"] = _bf((b1l[l] @ Wv[l].T).reshape(1, D))
    com["bq0"] = _f32((b1l[0] @ Wq[0].T).reshape(DT, 128).T)
    com["woT0"] = _bf(Wo[0].T)
    com["w1T0"] = _bf(_chunkR((W1[0] * g2[0][None, :]).T))
    com["b1T0"] = _f32((b1[0] + b2l[0] @ W1[0].T).reshape(FT, 128).T)
    com["w2T0"] = _bf(W2[0].T)
    com["b20"] = _bf(b2[0].reshape(1, D))
    com["bq1row"] = _f32((b1l[1] @ Wq[1].T).reshape(1, D))
    com["wo1T"] = _bf(Wo[1].T)
    com["b21row"] = _f32(b2[1].reshape(1, D))
    proj_eff = pW * outg[None, :]
    pbias_full = outb @ pW.T + pb
    b1_full_l2 = b1[1] + b2l[1] @ W1[1].T
    w1eff_l2 = W1[1] * g2[1][None, :]

    inv = 10000.0 ** (-np.arange(0, HD, 2, dtype=np.float64) / HD)
    posg = np.arange(N + 1, dtype=np.float64)
    ang = posg[None, :] * inv[:, None]
    cosl, sinl = np.cos(ang), np.sin(ang)
    cosl[:, 0], sinl[:, 0] = 1.0, 0.0
    cos64 = np.concatenate([cosl, cosl], 0)
    sinm64 = np.concatenate([-sinl, sinl], 0)
    cos128 = _f32(np.concatenate([cos64, cos64], 0))      # [128, N+1]
    sinm128 = _f32(np.concatenate([sinm64, sinm64], 0))

    in_maps = []
    for core in range(8):
        g, j = core // G, core % G
        kept_pos = np.concatenate(
            [[0], 1 + np.nonzero(mask[g] != 0)[0]]).astype(np.int64)
        nk = len(kept_pos)
        assert nk <= CAP, f"kept {nk} exceeds capacity {CAP}"
        pos_pad = np.zeros(CAP, np.int64)
        pos_pad[:nk] = kept_pos
        keep = np.zeros(CAP, np.float32)
        keep[:nk] = 1.0
        xcg = np.concatenate([pool, x[g]], axis=0)        # [N+1, D]
        xc_kept = np.zeros((CAP, D), np.float32)
        xc_kept[:nk] = xcg[kept_pos]

        sl = slice(j * TLOC, (j + 1) * TLOC)
        d = dict(com)
        d["x_sh"] = _f32(xc_kept[sl])
        d["cos_t"] = _f32(cos128[:, pos_pad[sl]])
        d["sinm_t"] = _f32(sinm128[:, pos_pad[sl]])
        d["keep_loc"] = _f32(keep[sl].reshape(NT, 128).T)
        d["keep_lb"] = _bf(keep[sl].reshape(NT, 128).T)
        d["keep_f8"] = keep[sl].reshape(NT, 128).T.astype(
            ml_dtypes.float8_e4m3)
        d["keep_all"] = _bf(keep.reshape(KT, 128).T)
        dffsl = slice(j * 1024, (j + 1) * 1024)
        d["w1T1s"] = _bf(w1eff_l2[dffsl, :].T)
        d["b1row"] = _f32(b1_full_l2[dffsl].reshape(1, D))
        d["w2T1s"] = _bf(W2[1][:, dffsl].T)
        osl = slice(j * 256, (j + 1) * 256)
        d["projTs"] = _bf(proj_eff[osl, :].T)
        d["pbias"] = _f32(pbias_full[osl].reshape(1, 256))
        in_maps.append(d)
    return in_maps


_PROGRAM = None
LAST = None  # last BassKernelResults (for test.py profiling)


def kernel(**inputs):
    global _PROGRAM, LAST
    from concourse.bass_utils import run_bass_kernel_spmd
    in_maps = _host_prep(inputs)
    if _PROGRAM is None:
        _PROGRAM = build_program()
    LAST = run_bass_kernel_spmd(_PROGRAM, in_maps, list(range(8)))
    res = LAST.results
    out = np.zeros((B, D), np.float32)
    for core in range(8):
        g, j = core // G, core % G
        out[g, j * 256:(j + 1) * 256] = np.asarray(
            res[core]["out"], np.float32).reshape(256)
    return out


# revision 31
# speedup vs baseline: 1.0574x; 1.0197x over previous
"""AttnPooling kernel for 8 TRN2 NeuronCores.

Key ideas vs the naive implementation:
- Host-side token compaction: masked tokens influence nothing (their keys
  are masked in layer 1, and only the pool row survives layer 2), so only
  unmasked tokens + pool are shipped. L: 2048 -> 1536 (capacity), exact.
- 2 batch groups x 4 sequence shards (384 tokens per core).
- Layer 1: K AllGathered in two chunks, then V, overlapping Q projection,
  RoPE and score/exp work with the collectives.
- Padding/mask folded into V: pad V rows are zeroed and the softmax
  denominator comes from a keep-flag column appended to V, so exp needs no
  bias and the math matches the -inf mask exactly.
- Scores computed transposed [keys, queries] with head pairs row-packed
  into the PE array (partitions 0-63 / 64-127 run concurrently).
- Layer 2 has no big collective: pool attention is a sum over keys, so
  each core reduces over its local K2/V2 and a 4 KB AllGather + local sum
  finishes softmax. MLP2 is DFF-sharded with a second 4 KB AllGather.
"""
import contextlib

import numpy as np
import ml_dtypes

BF16 = ml_dtypes.bfloat16
B, N, D = 2, 2047, 1024
H, HD = 16, 64
NL, DFF = 2, 4096
G = 4
CAP = 1536          # padded kept length (incl pool) per batch
TLOC = CAP // G     # 384 tokens per core
NT = TLOC // 128    # 3
DT = D // 128       # 8
KT = CAP // 128     # 12
FT = DFF // 128     # 32
KSH = 4 * 128 * TLOC          # K half-shard elems (4 feature tiles)
VSH = NT * 128 * (H * 65)     # V shard elems (vaug layout: per head v|keep)
ASH = VSH + KSH               # first AG shard: V(+ones) then K half 1
SM_SH = 2048                  # small AG shard (xn_pool 1024 + xc1p 1024) bf16
AT_SH = H * 65                # attn-partial AG shard (f32)
RG = [[0, 1, 2, 3], [4, 5, 6, 7]]
EPS = 1e-5


def _bf(a):
    return np.ascontiguousarray(np.asarray(a, np.float32)).astype(BF16)


def _f32(a):
    return np.ascontiguousarray(np.asarray(a, np.float32))


def _chunkR(wT):
    """[D, OUT] -> same-size array where chunk e ([128 out] x [D in]) is
    contiguous: R[e, p, d, j] = wT[d*128+p, e*128+j]."""
    Dd, OUT = wT.shape
    r = wT.reshape(Dd // 128, 128, OUT // 128, 128).transpose(2, 1, 0, 3)
    return np.ascontiguousarray(r).reshape(Dd, OUT)


def _f8(a):
    return np.ascontiguousarray(np.asarray(a, np.float32)).astype(
        ml_dtypes.float8_e4m3)


def build_program():
    import concourse.bass as bass
    import concourse.mybir as mybir
    import concourse.tile as tile

    f32 = mybir.dt.float32
    bf16 = mybir.dt.bfloat16

    nc = bass.Bass(num_devices=8)

    def din(name, shape, dt=bf16):
        return nc.declare_dram_parameter(name, shape, dt, isOutput=False)

    P = {}
    P["x_sh"] = din("x_sh", [TLOC, D], f32)
    P["ident"] = din("ident", [128, 128])
    P["cos_t"] = din("cos_t", [128, TLOC], f32)
    P["sinm_t"] = din("sinm_t", [128, TLOC], f32)
    P["keep_loc"] = din("keep_loc", [128, NT], f32)
    P["keep_lb"] = din("keep_lb", [128, NT])   # bf16 copy of keep_loc
    P["keep_f8"] = din("keep_f8", [128, NT], mybir.dt.float8e4)
    P["keep_all"] = din("keep_all", [128, KT])  # bf16, whole group
    for l in range(NL):
        for w in ("wq", "wk", "wv"):
            P[f"{w}T{l}"] = din(f"{w}T{l}", [D, D])
        P[f"bk{l}"] = din(f"bk{l}", [128, DT], f32)
        P[f"bv{l}"] = din(f"bv{l}", [1, D])
    P["bq0"] = din("bq0", [128, DT], f32)
    P["woT0"] = din("woT0", [D, D])
    P["w1T0"] = din("w1T0", [D, DFF])
    P["b1T0"] = din("b1T0", [128, FT], f32)
    P["w2T0"] = din("w2T0", [DFF, D])
    P["b20"] = din("b20", [1, D])
    P["bq1row"] = din("bq1row", [1, D], f32)
    P["wo1T"] = din("wo1T", [D, D])
    P["w1T1s"] = din("w1T1s", [D, D])
    P["b1row"] = din("b1row", [1, D], f32)
    P["w2T1s"] = din("w2T1s", [D, D])
    P["b21row"] = din("b21row", [1, D], f32)
    P["projTs"] = din("projTs", [D, 256])
    P["pbias"] = din("pbias", [1, 256], f32)
    P["out"] = nc.declare_dram_parameter("out", [1, 256], f32, isOutput=True)

    with tile.TileContext(nc) as tc:
        with contextlib.ExitStack() as es:
            _emit(nc, tc, es, P)
    _split_multiwaits(nc, mybir)
    return nc


def _split_multiwaits(nc, mybir):
    """Walrus caps sync commands on real compute ops; NoOps can hold many.
    Move multi-wait lists onto a NoOp inserted just before the instruction."""
    n = [0]

    def fresh():
        n[0] += 1
        return f"I-syncsplit-{n[0]}"

    for fn in nc.m.functions:
        for blk in fn.blocks:
            out = []
            for inst in blk.instructions:
                si = inst.sync_info
                if (si is not None and si.on_wait and len(si.on_wait) > 1
                        and type(inst).__name__ != "InstNoOp"):
                    for w in list(si.on_wait):
                        out.append(mybir.InstNoOp(
                            name=fresh(), ins=[], outs=[], engine=inst.engine,
                            sync_info=mybir.SyncInfo(on_wait=[w], on_update=[]),
                            bass_nofuse=True))
                    inst.sync_info = mybir.SyncInfo(
                        on_wait=[], on_update=list(si.on_update))
                out.append(inst)
            blk.instructions = out


def _emit(nc, tc, es, P):
    import concourse.bass as bass
    import concourse.mybir as mybir

    f32 = mybir.dt.float32
    bf16 = mybir.dt.bfloat16
    f8 = mybir.dt.float8e4
    AF = mybir.ActivationFunctionType
    OP = mybir.AluOpType
    AX = mybir.AxisListType
    ts = bass.ts
    ec = es.enter_context

    const = ec(tc.tile_pool(name="const", bufs=1))
    persist = ec(tc.tile_pool(name="persist", bufs=1))
    act = ec(tc.tile_pool(name="act", bufs=2))
    wbuf = ec(tc.tile_pool(name="wbuf", bufs=2))
    wpers = ec(tc.tile_pool(name="wpers", bufs=1))  # 8x [128,D] bf16, reused
    rope_p = ec(tc.tile_pool(name="rope", bufs=2))
    ptp = ec(tc.tile_pool(name="ptp", bufs=3))
    small = ec(tc.tile_pool(name="small", bufs=1))
    stats = ec(tc.tile_pool(name="stats", bufs=2))
    dram = ec(tc.tile_pool(name="dram", bufs=1, space="DRAM"))

    dma = nc.sync.dma_start

    def fview(dram_tile, off, p, f):
        a = dram_tile[:]
        return bass.AP(tensor=a.tensor, offset=a.offset + off,
                       ap=[[f, p], [1, f]])

    # ---------------- constants ----------------
    ident_sb = const.tile([128, 128], bf16, tag="ident", name="ident")
    dma(out=ident_sb[:], in_=P["ident"][:])
    cos_sb = const.tile([128, TLOC], f32, tag="cos", name="cos")
    dma(out=cos_sb[:], in_=P["cos_t"][:])
    sinm_sb = const.tile([128, TLOC], f32, tag="sinm", name="sinm")
    dma(out=sinm_sb[:], in_=P["sinm_t"][:])
    keep_sb = const.tile([128, NT], f32, tag="keep", name="keep")
    dma(out=keep_sb[:], in_=P["keep_loc"][:])
    eps_sb = const.tile([128, 1], f32, tag="eps", name="eps")
    nc.vector.memset(eps_sb[:], EPS)
    bq0_sb = const.tile([128, DT], f32, tag="bq0", name="bq0")
    dma(out=bq0_sb[:], in_=P["bq0"][:])
    bk_sb, bv_bc = {}, {}
    for l in range(NL):
        bk_sb[l] = const.tile([128, DT], f32, tag=f"bk{l}", name=f"bk{l}")
        dma(out=bk_sb[l][:], in_=P[f"bk{l}"][:])
        bv_bc[l] = const.tile([128, D], bf16, tag=f"bvbc{l}", name=f"bvbc{l}")
        dma(out=bv_bc[l][:], in_=P[f"bv{l}"][:].to_broadcast([128, D]))
    b1T0_sb = const.tile([128, FT], f32, tag="b1T0", name="b1T0")
    dma(out=b1T0_sb[:], in_=P["b1T0"][:])
    b2bc = const.tile([128, D], bf16, tag="b2bc", name="b2bc")
    dma(out=b2bc[:], in_=P["b20"][:].to_broadcast([128, D]))
    pbias_sb = const.tile([1, 256], f32, tag="pbias", name="pbias")
    dma(out=pbias_sb[:], in_=P["pbias"][:])
    bq1r_sb = const.tile([1, D], f32, tag="bq1r", name="bq1r")
    dma(out=bq1r_sb[:], in_=P["bq1row"][:])
    b1r_sb = const.tile([1, D], f32, tag="b1r", name="b1r")
    dma(out=b1r_sb[:], in_=P["b1row"][:])
    b21r_sb = const.tile([1, D], f32, tag="b21r", name="b21r")
    dma(out=b21r_sb[:], in_=P["b21row"][:])

    # residual stream f32, token-major
    x_res = [persist.tile([128, D], f32, tag=f"xres{t}", name=f"xres{t}")
             for t in range(NT)]
    for t in range(NT):
        dma(out=x_res[t][:], in_=P["x_sh"][ts(t, 128), :])

    # ---------------- helpers ----------------
    def ln_to_xnT(tags, psp):
        """LN of x_res (no affine; folded into weights) -> DT x [128,TLOC]
        bf16, feature-major (transposed)."""
        xnT = [persist.tile([128, TLOC], bf16, tag=tags[d], name=f"{tags[d]}_n")
               for d in range(DT)]
        for t in range(NT):
            st = stats.tile([128, 2, 6], f32, tag="bnst", name="bnst")
            nc.vector.bn_stats(out=st[:, 0, :], in_=x_res[t][:, 0:512])
            nc.vector.bn_stats(out=st[:, 1, :], in_=x_res[t][:, 512:1024])
            mv = stats.tile([128, 2], f32, tag="bnmv", name="bnmv")
            nc.vector.bn_aggr(out=mv[:], in_=st[:])
            std = stats.tile([128, 1], f32, tag="std", name="std")
            nc.scalar.activation(out=std[:], in_=mv[:, 1:2], func=AF.Sqrt,
                                 bias=eps_sb[:], scale=1.0)
            r = stats.tile([128, 1], f32, tag="rstd", name="rstd")
            nc.vector.reciprocal(out=r[:], in_=std[:])
            xsub = act.tile([128, D], f32, tag="xsub", name="xsub")
            nc.vector.tensor_scalar_sub(out=xsub[:], in0=x_res[t][:],
                                        scalar1=mv[:, 0:1])
            xn = act.tile([128, D], bf16, tag="xn", name="xn")
            nc.scalar.activation(out=xn[:], in_=xsub[:], func=AF.Identity,
                                 bias=0.0, scale=r[:])
            for d in range(DT):
                pt = psp.tile([128, 128], bf16, tag="tp", name="tp")
                nc.tensor.transpose(pt[:], xn[:, ts(d, 128)], ident_sb[:])
                nc.scalar.activation(out=xnT[d][:, ts(t, 128)], in_=pt[:],
                                     func=AF.Copy)
        return xnT

    def qk_proj(xnT, w_dram, bias_sb, tags, psp, rng=None, dest=None,
                odt=bf16):
        oT = dest if dest is not None else {}
        wa = w_dram[:]
        for e in (rng if rng is not None else range(DT)):
            if e not in oT:
                oT[e] = persist.tile([128, TLOC], odt, tag=tags[e],
                                     name=f"{tags[e]}_p")
            wc = wbuf.tile([128, DT, 128], bf16, tag="wchunk", name="wchunk")
            src = bass.AP(tensor=wa.tensor,
                          offset=wa.offset + e * 128 * DT * 128,
                          ap=[[DT * 128, 128], [128, DT], [1, 128]])
            dma(out=wc[:], in_=src)
            pq = psp.tile([128, 512], f32, tag="pq", name="pq")
            for d in range(DT):
                nc.tensor.matmul(pq[:, 0:TLOC], wc[:, d, :], xnT[d][:],
                                 start=(d == 0), stop=(d == DT - 1))
            nc.scalar.activation(out=oT[e][:], in_=pq[:, 0:TLOC],
                                 func=AF.Identity,
                                 bias=bias_sb[:, e:e + 1], scale=1.0)
        return oT

    def v_proj(xnT, w_dram, bvbc, tags, psh):
        """-> NT tiles [128, H*65] bf16 in vaug layout (per head v(64)|keep),
        bias added, pad rows zeroed, keep column from keep_lb."""
        v = [persist.tile([128, H * 65], bf16, tag=tags[t],
                          name=f"{tags[t]}_v") for t in range(NT)]
        for hlf in range(2):
            held = [psh.tile([128, 512], f32, tag=f"vh{i}", name=f"vh{i}")
                    for i in range(NT)]
            for d in range(DT):
                wvc = wbuf.tile([128, 512], bf16, tag="wvchunk", name="wvchunk")
                dma(out=wvc[:], in_=w_dram[ts(d, 128), ts(hlf, 512)])
                for t in range(NT):
                    nc.tensor.matmul(held[t][:], xnT[d][:, ts(t, 128)], wvc[:],
                                     start=(d == 0), stop=(d == DT - 1),
                                     skip_group_check=True)
            for t in range(NT):
                a = v[t][:]
                vout = bass.AP(tensor=a.tensor,
                               offset=a.offset + hlf * 8 * 65,
                               ap=[a.ap[0], [65, 8], [1, 64]])
                nc.vector.tensor_tensor(out=vout, in0=held[t][:],
                                        in1=bvbc[:, ts(hlf, 512)], op=OP.add)
                nc.vector.tensor_scalar_mul(out=vout, in0=vout,
                                            scalar1=keep_sb[:, t:t + 1])
        for t in range(NT):
            a = v[t][:]
            ones_dst = bass.AP(tensor=a.tensor, offset=a.offset + 64,
                               ap=[a.ap[0], [65, H]])
            kb = P["keep_lb"][:, t:t + 1]
            dma(out=ones_dst, in_=bass.AP(tensor=kb.tensor, offset=kb.offset,
                                          ap=[kb.ap[0], [0, H]]))
        return v

    def rope(q):
        shuf = rope_p.tile([128, TLOC], bf16, tag="shuf", name="shuf")
        for blk in range(2):
            b0 = 64 * blk
            nc.vector.tensor_copy(out=shuf[b0:b0 + 32, :],
                                  in_=q[b0 + 32:b0 + 64, :])
            nc.vector.tensor_copy(out=shuf[b0 + 32:b0 + 64, :],
                                  in_=q[b0:b0 + 32, :])
        qc = rope_p.tile([128, TLOC], bf16, tag="qcos", name="qcos")
        nc.vector.tensor_tensor(out=qc[:], in0=q[:], in1=cos_sb[:], op=OP.mult)
        qs = rope_p.tile([128, TLOC], bf16, tag="qsin", name="qsin")
        nc.vector.tensor_tensor(out=qs[:], in0=shuf[:], in1=sinm_sb[:],
                                op=OP.mult)
        nc.vector.tensor_tensor(out=q[:], in0=qc[:], in1=qs[:], op=OP.add)

    # ================= LAYER 1 =================
    xnT_tags = [f"xnT{d}" for d in range(DT)]
    qT_tags = [f"qT{d}" for d in range(DT)]
    kT_tags = [f"kT{d}" for d in range(DT)]
    vg_tags = [f"vg{t}" for t in range(NT)]

    with tc.tile_pool(name="ps_a", bufs=2, space="PSUM") as ps_a, \
         tc.tile_pool(name="ps_b", bufs=1, space="PSUM") as ps_b:
        xnT = ln_to_xnT(xnT_tags, ps_a)

        # V (vaug layout) + K half 1 in one AG, K half 2 in a second.
        v_sb = v_proj(xnT, P["wvT0"], bv_bc[0], vg_tags, ps_b)
        ag_a_in = dram.tile([ASH], bf16, tag="agai1", name="agai1")
        ag_a_out = dram.tile([G * ASH], bf16, tag="agao1", name="agao1")
        for t in range(NT):
            dma(out=fview(ag_a_in, t * 128 * H * 65, 128, H * 65),
                in_=v_sb[t][:])
        kT = {}
        qk_proj(xnT, P["wkT0"], bk_sb[0], kT_tags, ps_a, range(4), dest=kT)
        for e in range(4):
            rope(kT[e])
            dma(out=fview(ag_a_in, VSH + e * 128 * TLOC, 128, TLOC),
                in_=kT[e][:])
        nc.gpsimd.collective_compute(
            "AllGather", OP.bypass, replica_groups=RG,
            ins=[ag_a_in[:]], outs=[ag_a_out[:]])

        ag_b_in = dram.tile([KSH], bf16, tag="agbi", name="agbi")
        ag_b_out = dram.tile([G * KSH], bf16, tag="agbo", name="agbo")
        qk_proj(xnT, P["wkT0"], bk_sb[0], kT_tags, ps_a, range(4, 8), dest=kT)
        for e in range(4, 8):
            rope(kT[e])
            dma(out=fview(ag_b_in, (e - 4) * 128 * TLOC, 128, TLOC),
                in_=kT[e][:])
        nc.gpsimd.collective_compute(
            "AllGather", OP.bypass, replica_groups=RG,
            ins=[ag_b_in[:]], outs=[ag_b_out[:]])

        qT = qk_proj(xnT, P["wqT0"], bq0_sb, qT_tags, ps_a)
        for e in range(DT):
            rope(qT[e])

        # khd: [128, CAP] bf16 per feature tile; one 3D DMA each
        khd = []
        for dt_ in range(DT):
            t_ = persist.tile([128, CAP], bf16, tag=f"khd{dt_}",
                              name=f"khd{dt_}")
            khd.append(t_)
            if dt_ < 4:
                ao = ag_a_out[:]
                src = bass.AP(
                    tensor=ao.tensor,
                    offset=ao.offset + VSH + dt_ * 128 * TLOC,
                    ap=[[TLOC, 128], [ASH, G], [1, TLOC]])
            else:
                ao = ag_b_out[:]
                src = bass.AP(
                    tensor=ao.tensor,
                    offset=ao.offset + (dt_ - 4) * 128 * TLOC,
                    ap=[[TLOC, 128], [KSH, G], [1, TLOC]])
            dst = bass.AP(tensor=t_[:].tensor, offset=t_[:].offset,
                          ap=[t_[:].ap[0], [TLOC, G], [1, TLOC]])
            dma(out=dst, in_=src)

        # vaug[k]: contiguous copies from the gathered vaug-layout V
        vaug = []
        for k in range(KT):
            va = persist.tile([128, H * 65], bf16, tag=f"vg_a{k}",
                              name=f"vg_a{k}")
            vaug.append(va)
            r, lt = k // NT, k % NT
            dma(out=va[:],
                in_=fview(ag_a_out, r * ASH + lt * 128 * H * 65,
                          128, H * 65))

    # attention: scores transposed [keys, queries], head pairs row-packed
    oT = [persist.tile([128, TLOC], bf16, tag=xnT_tags[d], name=f"oT{d}")
          for d in range(DT)]
    # dens live 4-per-tile at partition starts {0,32,64,96} (engine ops
    # require 32-aligned start partitions)
    den_sb = [small.tile([128, TLOC], f32, tag=f"den{i}", name=f"den{i}")
              for i in range(4)]
    for i in range(4):
        nc.vector.memset(den_sb[i][:], 1.0)
    with tc.tile_pool(name="ps_sc", bufs=2, space="PSUM") as ps_sc, \
         tc.tile_pool(name="ps_pav", bufs=2, space="PSUM") as ps_pav:
        for dt_ in range(DT):
            pav = [ps_pav.tile([65, 512], f32, tag=f"pav{hh}",
                               name=f"pav{hh}_{dt_}") for hh in range(2)]
            pts = {}

            def _av(k):
                for hh in range(2):
                    h = 2 * dt_ + hh
                    nc.tensor.matmul(pav[hh][0:65, 0:TLOC],
                                     vaug[k][:, 65 * h:65 * h + 65],
                                     pts[k][:, hh, :],
                                     start=(k == 0), stop=(k == KT - 1),
                                     skip_group_check=True)

            for k in range(KT):
                sc = ps_sc.tile([128, 2, 512], f32, tag="sc", name="sc")
                for hh in range(2):
                    off = 64 * hh
                    nc.tensor.matmul(sc[:, hh, 0:TLOC],
                                     khd[dt_][off:off + 64, ts(k, 128)],
                                     qT[dt_][off:off + 64, :],
                                     start=True, stop=True,
                                     skip_group_check=True)
                pt = ptp.tile([128, 2, TLOC], bf16, tag="pt", name="pt")
                nc.scalar.activation(out=pt[:], in_=sc[:, :, 0:TLOC],
                                     func=AF.Exp, bias=0.0, scale=0.125)
                pts[k] = pt
                # pipeline: AV trails scores by one k so the in-order PE
                # queue never waits on the exp of the current tile
                if k >= 1:
                    _av(k - 1)
            _av(KT - 1)
            for hh in range(2):
                h = 2 * dt_ + hh
                off = 64 * hh
                nc.vector.tensor_copy(out=oT[dt_][off:off + 64, :],
                                      in_=pav[hh][0:64, 0:TLOC])
                dp = 32 * (h % 4)
                nc.vector.tensor_copy(
                    out=den_sb[h // 4][dp:dp + 1, :],
                    in_=pav[hh][64:65, 0:TLOC])

    invd_d = dram.tile([H * TLOC], f32, tag="invd_d", name="invd_d")
    for i in range(4):
        inv4 = small.tile([128, TLOC], f32, tag="inv4", name=f"inv4_{i}")
        nc.vector.reciprocal(out=inv4[:], in_=den_sb[i][:])
        a = inv4[:]
        src = bass.AP(tensor=a.tensor, offset=a.offset,
                      ap=[[a.ap[0][0] * 32, 4], [1, TLOC]])
        dma(out=fview(invd_d, 4 * i * TLOC, 4, TLOC), in_=src)
    for dt_ in range(DT):
        bc = act.tile([128, TLOC], f32, tag="invdbc", name="invdbc")
        ia = invd_d[:]
        src = bass.AP(tensor=ia.tensor, offset=ia.offset + 2 * dt_ * TLOC,
                      ap=[[TLOC, 2], [0, 64], [1, TLOC]])
        dma(out=bc[:], in_=src)
        nc.vector.tensor_tensor(out=oT[dt_][:], in0=oT[dt_][:], in1=bc[:],
                                op=OP.mult)

    # O-projection + residual
    w8 = [wpers.tile([128, D], bf16, tag=f"w8_{d}", name=f"wo8_{d}")
          for d in range(DT)]
    for d in range(DT):
        dma(out=w8[d][:], in_=P["woT0"][ts(d, 128), :])
    with tc.tile_pool(name="ps_c", bufs=2, space="PSUM") as ps_c:
        for t in range(NT):
            for hlf in range(2):
                po = ps_c.tile([128, 512], f32, tag="po", name="po")
                for d in range(DT):
                    nc.tensor.matmul(po[:], oT[d][:, ts(t, 128)],
                                     w8[d][:, ts(hlf, 512)],
                                     start=(d == 0), stop=(d == DT - 1))
                nc.vector.tensor_tensor(out=x_res[t][:, ts(hlf, 512)],
                                        in0=x_res[t][:, ts(hlf, 512)],
                                        in1=po[:], op=OP.add)

    # tail-weight prefetch into now-dead SBUF: wq1 -> w8 (during MLP1),
    # w1T1s -> khd slots, w2T1s -> vg_a slots, proj -> its own tiles
    for d in range(DT):
        dma(out=w8[d][:], in_=P["wqT1"][ts(d, 128), :])
    w1s_sb = [persist.tile([128, 512], bf16, tag=f"khd{d}", name=f"w1s{d}")
              for d in range(DT)]
    w1s_sb += [persist.tile([128, 512], bf16, tag=f"vg_a{k}",
                            name=f"w1sb{k}") for k in range(3, 11)]
    for d in range(DT):
        dma(out=w1s_sb[d][:], in_=P["w1T1s"][ts(d, 128), 0:512])
        dma(out=w1s_sb[8 + d][:], in_=P["w1T1s"][ts(d, 128), 512:1024])

    proj_sb = [wbuf.tile([128, 256], bf16, tag=f"projc{d}", name=f"projc{d}")
               for d in range(DT)]
    for d in range(DT):
        dma(out=proj_sb[d][:], in_=P["projTs"][ts(d, 128), :])

    with tc.tile_pool(name="ps_d", bufs=2, space="PSUM") as ps_d:
        xn2T = ln_to_xnT(qT_tags, ps_d)  # reuse qT slots (dead)
    for t in range(NT):  # pre-add the MLP output bias while PE runs MLP
        nc.vector.tensor_tensor(out=x_res[t][:], in0=x_res[t][:],
                                in1=b2bc[:], op=OP.add)

    with tc.tile_pool(name="ps_mlp", bufs=2, space="PSUM") as ps_mlp, \
         tc.tile_pool(name="ps_hld", bufs=1, space="PSUM") as ps_hld:
        held = [ps_hld.tile([128, 2, 512], f32, tag=f"mh{t}", name=f"mh{t}")
                for t in range(NT)]
        hTs, w2cs = {}, {}

        def _mlp2nd(f):
            for t in range(NT):
                for hlf in range(2):
                    nc.tensor.matmul(held[t][:, hlf, :], hTs[f][:, ts(t, 128)],
                                     w2cs[f][:, ts(hlf, 512)],
                                     start=(f == 0), stop=(f == FT - 1),
                                     skip_group_check=True)

        for f in range(FT):
            wc = wbuf.tile([128, DT, 128], bf16, tag="w1chunk", name="w1chunk")
            wa = P["w1T0"][:]
            src = bass.AP(tensor=wa.tensor,
                          offset=wa.offset + f * 128 * DT * 128,
                          ap=[[DT * 128, 128], [128, DT], [1, 128]])
            dma(out=wc[:], in_=src)
            ph = ps_mlp.tile([128, 512], f32, tag="ph", name="ph")
            for d in range(DT):
                nc.tensor.matmul(ph[:, 0:TLOC], wc[:, d, :], xn2T[d][:],
                                 start=(d == 0), stop=(d == DT - 1))
            hT = act.tile([128, TLOC], bf16, tag="hT", name="hT")
            nc.scalar.activation(out=hT[:], in_=ph[:, 0:TLOC], func=AF.Gelu,
                                 bias=b1T0_sb[:, f:f + 1], scale=1.0)
            hTs[f] = hT
            w2c = wbuf.tile([128, D], bf16, tag="w2chunk", name="w2chunk")
            dma(out=w2c[:], in_=P["w2T0"][ts(f, 128), :])
            w2cs[f] = w2c
            if f >= 1:
                _mlp2nd(f - 1)
        _mlp2nd(FT - 1)
        for t in range(NT):
            for hlf in range(2):
                nc.vector.tensor_tensor(out=x_res[t][:, ts(hlf, 512)],
                                        in0=x_res[t][:, ts(hlf, 512)],
                                        in1=held[t][:, hlf, :], op=OP.add)

    # ================= LAYER 2 =================
    with tc.tile_pool(name="ps_e", bufs=2, space="PSUM") as ps_e:
        xn3T = ln_to_xnT(xnT_tags, ps_e)

        # small AG: pool xn (feature-major col 0) + pool residual row, bf16
        xc1p_bf = small.tile([1, D], bf16, tag="rb_a", name="xc1pbf")
        nc.scalar.activation(out=xc1p_bf[:], in_=x_res[0][0:1, :], func=AF.Copy)
        ag_s_in = dram.tile([SM_SH], bf16, tag="agsi", name="agsi")
        ag_s_out = dram.tile([G * SM_SH], bf16, tag="agso", name="agso")
        pcol = small.tile([128, DT], bf16, tag="pcol", name="pcol")
        for d in range(DT):
            nc.vector.tensor_copy(out=pcol[:, d:d + 1], in_=xn3T[d][:, 0:1])
        ai = ag_s_in[:]
        dma(out=bass.AP(tensor=ai.tensor, offset=ai.offset,
                        ap=[[1, 128], [128, DT]]), in_=pcol[:])
        dma(out=fview(ag_s_in, D, 1, D), in_=xc1p_bf[:])
        nc.gpsimd.collective_compute(
            "AllGather", OP.bypass, replica_groups=RG,
            ins=[ag_s_in[:]], outs=[ag_s_out[:]])

        kT2 = qk_proj(xn3T, P["wkT1"], bk_sb[1], kT_tags, ps_e)
        for e in range(DT):
            rope(kT2[e])

    with tc.tile_pool(name="ps_f", bufs=1, space="PSUM") as ps_f:
        v2 = v_proj(xn3T, P["wvT1"], bv_bc[1], vg_tags, ps_f)

    vaug2 = v2  # v_proj already emits the vaug layout with keep column

    with tc.tile_pool(name="ps_g", bufs=1, space="PSUM") as ps:
        # read back pool xn + residual from shard 0 of small AG
        xnp = small.tile([128, DT], bf16, tag="xnp", name="xnp")
        ao = ag_s_out[:]
        dma(out=xnp[:], in_=bass.AP(tensor=ao.tensor, offset=ao.offset,
                                    ap=[[1, 128], [128, DT]]))
        xc1p = small.tile([1, D], f32, tag="rf_a", name="xc1p")
        xc1p_b2 = small.tile([1, D], bf16, tag="rb_b", name="xc1pb2")
        dma(out=xc1p_b2[:], in_=bass.AP(tensor=ao.tensor,
                                        offset=ao.offset + D,
                                        ap=[[1, 1], [1, D]]))
        nc.vector.tensor_copy(out=xc1p[:], in_=xc1p_b2[:])

        # q2 row = xnp.T @ Wq1T + bias (w8 holds prefetched Wq1)
        rps = ps.tile([1, D], f32, tag="rps", name="q2ps")
        for hlf in range(2):
            for d in range(DT):
                nc.tensor.matmul(rps[:, ts(hlf, 512)], xnp[:, d:d + 1],
                                 w8[d][:, ts(hlf, 512)],
                                 start=(d == 0), stop=(d == DT - 1),
                                 skip_group_check=True)
        for d in range(DT):  # wo1 -> w8; overlaps partial attention
            dma(out=w8[d][:], in_=P["wo1T"][ts(d, 128), :])
        q2row = small.tile([1, D], bf16, tag="rb_c", name="q2row")
        q2f = small.tile([1, D], f32, tag="rf_b", name="q2f")
        nc.vector.tensor_tensor(out=q2f[:], in0=rps[:], in1=bq1r_sb[:],
                                op=OP.add)
        nc.vector.tensor_copy(out=q2row[:], in_=q2f[:])
        q2_d = dram.tile([D], bf16, tag="q2d", name="q2d")
        dma(out=fview(q2_d, 0, 1, D), in_=q2row[:])
        q2T = small.tile([128, DT], bf16, tag="q2T", name="q2T")
        qd = q2_d[:]
        dma(out=q2T[:], in_=bass.AP(tensor=qd.tensor, offset=qd.offset,
                                    ap=[[1, 128], [128, DT]]))

        # partial pool attention over local keys
        p2ps = ps.tile([128, 2 * NT * DT], f32, tag="p2ps", name="p2ps")
        for dt_ in range(DT):
            for k in range(NT):
                for hh in range(2):
                    off = 64 * hh
                    c = dt_ * 2 * NT + hh * NT + k
                    nc.tensor.matmul(p2ps[:, c:c + 1],
                                     kT2[dt_][off:off + 64, ts(k, 128)],
                                     q2T[off:off + 64, dt_:dt_ + 1],
                                     start=True, stop=True,
                                     skip_group_check=True)
        p2sb = small.tile([128, 2 * NT * DT], bf16, tag="p2sb", name="p2sb")
        nc.scalar.activation(out=p2sb[:], in_=p2ps[:], func=AF.Exp,
                             bias=0.0, scale=0.125)
        # h outer / k inner: accumulation groups must be sequential within a
        # PSUM bank (start=True clears the whole bank's has_written bits)
        o2ps = ps.tile([65, 16], f32, tag="o2ps", name="o2ps")
        for h in range(H):
            for k in range(NT):
                c = (h // 2) * 2 * NT + (h % 2) * NT + k
                nc.tensor.matmul(o2ps[:, h:h + 1],
                                 vaug2[k][:, 65 * h:65 * h + 65],
                                 p2sb[:, c:c + 1],
                                 start=(k == 0), stop=(k == NT - 1),
                                 skip_group_check=True)
        # partials -> AG -> sum
        part_sb = small.tile([65, 16], f32, tag="part", name="part")
        nc.vector.tensor_copy(out=part_sb[:], in_=o2ps[:])
        ag_a_in = dram.tile([AT_SH], f32, tag="agai", name="agai")
        ag_a_out = dram.tile([G * AT_SH], f32, tag="agao", name="agao")
        dma(out=fview(ag_a_in, 0, 65, 16), in_=part_sb[:])
        nc.gpsimd.collective_compute(
            "AllGather", OP.bypass, replica_groups=RG,
            ins=[ag_a_in[:]], outs=[ag_a_out[:]])
        sums = small.tile([65, G, 16], f32, tag="sums", name="sums")
        for r in range(G):
            dma(out=sums[:, r, :], in_=fview(ag_a_out, r * AT_SH, 65, 16))
        tot = small.tile([65, 16], f32, tag="tot", name="tot")
        nc.vector.tensor_tensor(out=sums[:, 0, :], in0=sums[:, 0, :],
                                in1=sums[:, 1, :], op=OP.add)
        nc.vector.tensor_tensor(out=sums[:, 2, :], in0=sums[:, 2, :],
                                in1=sums[:, 3, :], op=OP.add)
        nc.vector.tensor_tensor(out=tot[:], in0=sums[:, 0, :],
                                in1=sums[:, 2, :], op=OP.add)
        # o2 feature-major [128, DT] f32 via 2 DMAs; divide -> bf16
        o2f = small.tile([128, DT], f32, tag="o2f", name="o2f")
        ta = tot[:]
        for a in range(2):
            dma(out=o2f[64 * a:64 * a + 64, :],
                in_=bass.AP(tensor=ta.tensor, offset=ta.offset + a,
                            ap=[[ta.ap[0][0], 64], [2, DT]]))
        den2 = small.tile([1, H], f32, tag="den2", name="den2")
        nc.vector.tensor_copy(out=den2[:], in_=tot[64:65, :])
        invd2 = small.tile([1, H], f32, tag="invd2", name="invd2")
        nc.vector.reciprocal(out=invd2[:], in_=den2[:])
        den_d = dram.tile([H], f32, tag="den_d", name="den_d")
        dma(out=fview(den_d, 0, 1, H), in_=invd2[:])
        o2bc = small.tile([128, DT], f32, tag="o2bc", name="o2bc")
        dd = den_d[:]
        for a in range(2):
            dma(out=o2bc[64 * a:64 * a + 64, :],
                in_=bass.AP(tensor=dd.tensor, offset=dd.offset + a,
                            ap=[[0, 64], [2, DT]]))
        o2sb = small.tile([128, DT], bf16, tag="o2sb", name="o2sb")
        nc.vector.tensor_tensor(out=o2sb[:], in0=o2f[:], in1=o2bc[:],
                                op=OP.mult)

        # x2 row = xc1p + o2 @ Wo2 (w8 holds prefetched Wo1)
        rps2 = ps.tile([1, D], f32, tag="rps", name="x2ps")
        for hlf in range(2):
            for d in range(DT):
                nc.tensor.matmul(rps2[:, ts(hlf, 512)], o2sb[:, d:d + 1],
                                 w8[d][:, ts(hlf, 512)],
                                 start=(d == 0), stop=(d == DT - 1),
                                 skip_group_check=True)
        for d in range(DT):  # w2T1s -> w8; overlaps rowLN/gelu below
            dma(out=w8[d][:], in_=P["w2T1s"][ts(d, 128), :])
        x2row = small.tile([1, D], f32, tag="rf_c", name="x2row")
        nc.vector.tensor_tensor(out=x2row[:], in0=xc1p[:], in1=rps2[:],
                                op=OP.add)

        def row_ln(xrow, out_tag, nm):
            """LN of a [1, D] f32 row -> [1, D] bf16 (no affine)."""
            s1 = small.tile([1, 1], f32, tag="lns1", name=f"{nm}s1")
            nc.vector.reduce_sum(out=s1[:], in_=xrow[:], axis=AX.X)
            mean = small.tile([1, 1], f32, tag="lnmean", name=f"{nm}mean")
            nc.vector.tensor_scalar_mul(out=mean[:], in0=s1[:],
                                        scalar1=1.0 / D)
            xc = small.tile([1, D], f32, tag="rf_d", name=f"{nm}xc")
            nc.vector.tensor_scalar_sub(out=xc[:], in0=xrow[:],
                                        scalar1=mean[:])
            sq = small.tile([1, D], f32, tag="rf_e", name=f"{nm}sq")
            nc.vector.tensor_tensor(out=sq[:], in0=xc[:], in1=xc[:],
                                    op=OP.mult)
            s2 = small.tile([1, 1], f32, tag="lns2", name=f"{nm}s2")
            nc.vector.reduce_sum(out=s2[:], in_=sq[:], axis=AX.X)
            std = small.tile([1, 1], f32, tag="lnstd", name=f"{nm}sd")
            nc.scalar.activation(out=std[:], in_=s2[:], func=AF.Sqrt,
                                 bias=eps_sb[0:1, :], scale=1.0 / D)
            rr = small.tile([1, 1], f32, tag="lnrr", name=f"{nm}rr")
            nc.vector.reciprocal(out=rr[:], in_=std[:])
            xo = small.tile([1, D], bf16, tag=out_tag, name=f"{nm}o")
            nc.scalar.activation(out=xo[:], in_=xc[:], func=AF.Identity,
                                 bias=0.0, scale=rr[:])
            return xo

        def row_to_fmaj(row_bf, tag, nm):
            """[1, D] bf16 row -> [128, DT] bf16 feature-major via DRAM."""
            rd = dram.tile([D], bf16, tag=f"{tag}_d", name=f"{nm}_d")
            dma(out=fview(rd, 0, 1, D), in_=row_bf[:])
            fm = small.tile([128, DT], bf16, tag=tag, name=nm)
            a = rd[:]
            dma(out=fm[:], in_=bass.AP(tensor=a.tensor, offset=a.offset,
                                       ap=[[1, 128], [128, DT]]))
            return fm

        xn2f = row_ln(x2row, "rb_d", "ln2f")
        xn2fm = row_to_fmaj(xn2f, "fm_a", "xn2fm")

        # sharded MLP2 (this core's 1024 DFF rows), weights prefetched
        hps = ps.tile([1, D], f32, tag="rps", name="hps")
        for hlf in range(2):
            for d in range(DT):
                nc.tensor.matmul(hps[:, ts(hlf, 512)], xn2fm[:, d:d + 1],
                                 w1s_sb[8 * hlf + d][:],
                                 start=(d == 0), stop=(d == DT - 1),
                                 skip_group_check=True)
        hrow_f = small.tile([1, D], f32, tag="rf_b", name="hrowf")
        nc.vector.tensor_tensor(out=hrow_f[:], in0=hps[:], in1=b1r_sb[:],
                                op=OP.add)
        hrow = small.tile([1, D], bf16, tag="rb_c", name="hrow")
        nc.scalar.activation(out=hrow[:], in_=hrow_f[:], func=AF.Gelu,
                             bias=0.0, scale=1.0)
        hfm = row_to_fmaj(hrow, "fm_b", "hfm")
        yps = ps.tile([1, D], f32, tag="rps", name="yps")
        for hlf in range(2):
            for d in range(DT):
                nc.tensor.matmul(yps[:, ts(hlf, 512)], hfm[:, d:d + 1],
                                 w8[d][:, ts(hlf, 512)],
                                 start=(d == 0), stop=(d == DT - 1),
                                 skip_group_check=True)
        y2row = small.tile([1, D], f32, tag="rf_b", name="y2row")
        nc.vector.tensor_copy(out=y2row[:], in_=yps[:])
        ag_m_in = dram.tile([D], f32, tag="agmi", name="agmi")
        ag_m_out = dram.tile([G * D], f32, tag="agmo", name="agmo")
        dma(out=fview(ag_m_in, 0, 1, D), in_=y2row[:])
        nc.gpsimd.collective_compute(
            "AllGather", OP.bypass, replica_groups=RG,
            ins=[ag_m_in[:]], outs=[ag_m_out[:]])
        yacc = small.tile([1, D], f32, tag="rf_d", name="yacc")
        dma(out=yacc[:], in_=fview(ag_m_out, 0, 1, D))
        for r in range(1, G):
            ypart = small.tile([1, D], f32, tag="rf_e", name=f"ypart{r}")
            dma(out=ypart[:], in_=fview(ag_m_out, r * D, 1, D))
            nc.vector.tensor_tensor(out=yacc[:], in0=yacc[:], in1=ypart[:],
                                    op=OP.add)
        x3row = small.tile([1, D], f32, tag="rf_a", name="x3row")
        nc.vector.tensor_tensor(out=x3row[:], in0=x2row[:], in1=yacc[:],
                                op=OP.add)
        nc.vector.tensor_tensor(out=x3row[:], in0=x3row[:], in1=b21r_sb[:],
                                op=OP.add)

        xn3 = row_ln(x3row, "rb_d", "ln3")
        xn3fm = row_to_fmaj(xn3, "fm_a", "xn3fm")
        pps = ps.tile([1, 256], f32, tag="pps", name="pps")
        for d in range(DT):
            nc.tensor.matmul(pps[:], xn3fm[:, d:d + 1], proj_sb[d][:],
                             start=(d == 0), stop=(d == DT - 1),
                             skip_group_check=True)
        outsb = small.tile([1, 256], f32, tag="rf_e", name="outsb")
        nc.vector.tensor_tensor(out=outsb[:], in0=pps[:], in1=pbias_sb[:],
                                op=OP.add)
        dma(out=P["out"][:], in_=outsb[:])


def _host_prep(inputs):
    x = _f32(inputs["x"])
    mask = np.asarray(inputs["attention_mask"])
    pool = _f32(inputs["pool_token"]).reshape(1, D)

    Wq, Wk, Wv, Wo = (_f32(inputs[k]) for k in ("Wq", "Wk", "Wv", "Wo"))
    g1, b1l = _f32(inputs["ln1_g"]), _f32(inputs["ln1_b"])
    g2, b2l = _f32(inputs["ln2_g"]), _f32(inputs["ln2_b"])
    W1, b1 = _f32(inputs["W1"]), _f32(inputs["b1"])
    W2, b2 = _f32(inputs["W2"]), _f32(inputs["b2"])
    outg, outb = _f32(inputs["out_g"]), _f32(inputs["out_b"])
    pW, pb = _f32(inputs["proj_W"]), _f32(inputs["proj_b"])

    com = {"ident": _bf(np.eye(128))}
    for l in range(NL):
        wq_c = _chunkR((Wq[l] * g1[l][None, :]).T) if l == 0 else \
            (Wq[l] * g1[l][None, :]).T
        com[f"wqT{l}"] = _bf(wq_c)
        com[f"wkT{l}"] = _bf(_chunkR((Wk[l] * g1[l][None, :]).T))
        com[f"wvT{l}"] = _bf((Wv[l] * g1[l][None, :]).T)
        com[f"bk{l}"] = _f32((b1l[l] @ Wk[l].T).reshape(DT, 128).T)
        com[f"bv{l}"] = _bf((b1l[l] @ Wv[l].T).reshape(1, D))
    com["bq0"] = _f32((b1l[0] @ Wq[0].T).reshape(DT, 128).T)
    com["woT0"] = _bf(Wo[0].T)
    com["w1T0"] = _bf(_chunkR((W1[0] * g2[0][None, :]).T))
    com["b1T0"] = _f32((b1[0] + b2l[0] @ W1[0].T).reshape(FT, 128).T)
    com["w2T0"] = _bf(W2[0].T)
    com["b20"] = _bf(b2[0].reshape(1, D))
    com["bq1row"] = _f32((b1l[1] @ Wq[1].T).reshape(1, D))
    com["wo1T"] = _bf(Wo[1].T)
    com["b21row"] = _f32(b2[1].reshape(1, D))
    proj_eff = pW * outg[None, :]
    pbias_full = outb @ pW.T + pb
    b1_full_l2 = b1[1] + b2l[1] @ W1[1].T
    w1eff_l2 = W1[1] * g2[1][None, :]

    inv = 10000.0 ** (-np.arange(0, HD, 2, dtype=np.float64) / HD)
    posg = np.arange(N + 1, dtype=np.float64)
    ang = posg[None, :] * inv[:, None]
    cosl, sinl = np.cos(ang), np.sin(ang)
    cosl[:, 0], sinl[:, 0] = 1.0, 0.0
    cos64 = np.concatenate([cosl, cosl], 0)
    sinm64 = np.concatenate([-sinl, sinl], 0)
    cos128 = _f32(np.concatenate([cos64, cos64], 0))      # [128, N+1]
    sinm128 = _f32(np.concatenate([sinm64, sinm64], 0))

    in_maps = []
    for core in range(8):
        g, j = core // G, core % G
        kept_pos = np.concatenate(
            [[0], 1 + np.nonzero(mask[g] != 0)[0]]).astype(np.int64)
        nk = len(kept_pos)
        assert nk <= CAP, f"kept {nk} exceeds capacity {CAP}"
        pos_pad = np.zeros(CAP, np.int64)
        pos_pad[:nk] = kept_pos
        keep = np.zeros(CAP, np.float32)
        keep[:nk] = 1.0
        xcg = np.concatenate([pool, x[g]], axis=0)        # [N+1, D]
        xc_kept = np.zeros((CAP, D), np.float32)
        xc_kept[:nk] = xcg[kept_pos]

        sl = slice(j * TLOC, (j + 1) * TLOC)
        d = dict(com)
        d["x_sh"] = _f32(xc_kept[sl])
        d["cos_t"] = _f32(cos128[:, pos_pad[sl]])
        d["sinm_t"] = _f32(sinm128[:, pos_pad[sl]])
        d["keep_loc"] = _f32(keep[sl].reshape(NT, 128).T)
        d["keep_lb"] = _bf(keep[sl].reshape(NT, 128).T)
        d["keep_f8"] = keep[sl].reshape(NT, 128).T.astype(
            ml_dtypes.float8_e4m3)
        d["keep_all"] = _bf(keep.reshape(KT, 128).T)
        dffsl = slice(j * 1024, (j + 1) * 1024)
        d["w1T1s"] = _bf(w1eff_l2[dffsl, :].T)
        d["b1row"] = _f32(b1_full_l2[dffsl].reshape(1, D))
        d["w2T1s"] = _bf(W2[1][:, dffsl].T)
        osl = slice(j * 256, (j + 1) * 256)
        d["projTs"] = _bf(proj_eff[osl, :].T)
        d["pbias"] = _f32(pbias_full[osl].reshape(1, 256))
        in_maps.append(d)
    return in_maps


_PROGRAM = None
LAST = None  # last BassKernelResults (for test.py profiling)


def kernel(**inputs):
    global _PROGRAM, LAST
    from concourse.bass_utils import run_bass_kernel_spmd
    in_maps = _host_prep(inputs)
    if _PROGRAM is None:
        _PROGRAM = build_program()
    LAST = run_bass_kernel_spmd(_PROGRAM, in_maps, list(range(8)))
    res = LAST.results
    out = np.zeros((B, D), np.float32)
    for core in range(8):
        g, j = core // G, core % G
        out[g, j * 256:(j + 1) * 256] = np.asarray(
            res[core]["out"], np.float32).reshape(256)
    return out
